# revision 1
# baseline (speedup 1.0000x reference)
"""AMPNN (gnn_message_passing) distributed Bass kernel for 8 TRN2 cores.

Strategy (matches the problem's sharding hint): column-shard the
[n_node, n_edge] incidence matrix over edges, run the masked softmax +
aggregation as TensorE matmuls against the bf16 incidence shard held in
SBUF, and AllReduce the per-node context.

node_edge_mask = where(ne>0, 0, -1e6) makes the reference softmax an exact
per-node softmax over incident edges (non-incident terms underflow to 0),
so per edge we compute p = exp(a) and accumulate
  ctx_raw[:, n] = sum_e (p*hid)[e, :] * ne[n, e]      (128 x n matmul)
  Z[n]          = sum_e p[e] * ne[n, e]               (1 x n matmul)
with ne_shard^T [1024, 4096] as the moving operand (bf16, exact for 0/1).

- edges sharded 1024/core; nodes 512/core; mols 32/core
- gather h[us], h[vs] via gpsimd.dma_gather from a bf16 node table in DRAM
  (layer 0 gathers padded node_features instead and applies the input MLP
  per-edge: lrelu/gather commute); PE transposes to channel-partition
- per-edge MLPs as TensorE matmuls with channels on partitions (bf16)
- the accumulator is laid out as 8 row-blocks of [129, 513] (ctx^T
  channels + Z row, one block per node stripe, + an S column for the
  isolated-node uniform-softmax fallback), so ONE ReduceScatter per layer
  hands each core exactly its stripe with no core-dependent addressing
- the ReduceScattered ctx^T block is directly the GRU's lhsT (no
  transposes);
  normalization by 1/Z and the isolated-node fallback are applied to the
  matmul output with per-partition scales / rank-1 terms
- GRU on the local node stripe (bf16 matmuls, f32 gates/state); relu'd h
  AllGathered in bf16 as the next layer's gather table
- attentive pooling per mol stripe (16 contiguous nodes/mol); host embeds
  the [32, 512] wts band into the exact-zero [256, 4096] output
"""
import os
import numpy as np
import ml_dtypes
import concourse.bass as bass
import concourse.bacc as bacc
import concourse.tile as tile
import concourse.mybir as mybir

F32 = mybir.dt.float32
BF16 = mybir.dt.bfloat16
I16 = mybir.dt.int16
AF = mybir.ActivationFunctionType
ALU = mybir.AluOpType
ts = bass.ts

N_CORES = 8
N, E, M = 4096, 8192, 256
ND, ED, H, HE, C, L = 64, 16, 128, 64, 128, 3
ES = E // N_CORES          # 1024 edges/core
NS = N // N_CORES          # 512 nodes/core
MS = M // N_CORES          # 32 mols/core
ET = ES // 128             # 8 edge tiles
NT = NS // 128             # 4 node tiles
AC_R = 129                 # block rows: 128 ctx channels + Z row
AC_C = NS + 1              # block cols: 512 stripe nodes + S column
RG = [list(range(N_CORES))]


def wrap16(a):
    """[n] -> [128, n//16]: 16-partition wrap (unwrapped[i] ==
    wrapped[i % 16, i // 16]) replicated to all 128 partitions (the 8
    gpsimd Q7 cores each read their own 16-partition copy)."""
    w = np.ascontiguousarray(a.reshape(-1, 16).T)
    return np.tile(w, (8, 1))


def bf(x):
    return np.ascontiguousarray(x).astype(ml_dtypes.bfloat16)


def prep_inputs(inputs):
    """Full problem inputs -> per-core in_maps (host-side shard/layout)."""
    nf = np.asarray(inputs["node_features"], np.float32)
    ef = np.asarray(inputs["edge_features"], np.float32)
    us = np.asarray(inputs["us"], np.int64)
    vs = np.asarray(inputs["vs"], np.int64)
    mn = np.asarray(inputs["mol_node_matrix"], np.float32)
    ne = np.asarray(inputs["node_edge_matrix"], np.float32)
    p = {k: np.asarray(inputs[k], np.float32) for k in (
        "fcn_w", "fcn_b", "fce_w", "fce_b", "m_w", "m_b", "me_w", "me_b",
        "ma_w", "ma_b", "g_wih", "g_whh", "g_bih", "g_bhh",
        "pool_at_w", "pool_at_b", "pool_w", "pool_b")}

    nfpad = np.zeros((N, 128), np.float32)
    nfpad[:, :ND] = nf
    nfpad[:, ND] = 1.0

    fcnwT = np.concatenate([p["fcn_w"].T, p["fcn_b"][None, :]], 0)   # [65,128]
    fcewT = np.concatenate([p["fce_w"].T, p["fce_b"][None, :]], 0)   # [17,64]
    hs = np.hstack
    mwu = hs([p["m_w"][i][:, 0:H].T for i in range(L)])              # [128,3*128]
    mwv = hs([p["m_w"][i][:, H:2 * H].T for i in range(L)])
    mwe = hs([np.concatenate([p["m_w"][i][:, 2 * H:].T,
                              p["m_b"][i][None, :]], 0) for i in range(L)])
    mewu = hs([p["me_w"][i][:, 0:H].T for i in range(L)])            # [128,3*64]
    mewv = hs([p["me_w"][i][:, H:2 * H].T for i in range(L)])
    mewe = hs([np.concatenate([p["me_w"][i][:, 2 * H:].T,
                               p["me_b"][i][None, :]], 0) for i in range(L)])
    gwih = hs([p["g_wih"][i].T for i in range(L)])                   # [128,3*384]
    gwhh = hs([p["g_whh"][i].T for i in range(L)])
    # r,z bias slices can ride on the h-side rank-1 bias matmul; the n-slice
    # of g_bih must be added after the r*hn product, so it goes separately.
    gbh = np.concatenate([
        np.concatenate([p["g_bih"][i][0:2 * H] + p["g_bhh"][i][0:2 * H],
                        p["g_bhh"][i][2 * H:]]) for i in range(L)])
    gbin = hs([p["g_bih"][i][None, 2 * H:] for i in range(L)])       # [1,3*128]

    common = {
        "nfpad": bf(nfpad),
        "fcnwT": bf(fcnwT), "fcewT": bf(fcewT),
        "mwu": bf(mwu), "mwv": bf(mwv), "mwe": bf(mwe),
        "mewu": bf(mewu), "mewv": bf(mewv), "mewe": bf(mewe),
        "maw": np.ascontiguousarray(p["ma_w"].reshape(1, L * C)),
        "mab": np.ascontiguousarray(p["ma_b"].reshape(1, L)),
        "gwih": bf(gwih), "gwhh": bf(gwhh),
        "gbh": bf(gbh.reshape(1, L * 3 * H)),
        "gbin": np.ascontiguousarray(gbin),                          # f32
        "paw": bf(p["pool_at_w"].T),                                 # [128,1]
        "pab": p["pool_at_b"].reshape(1, 1).astype(np.float32),
        "pw": bf(p["pool_w"].T),                                     # [128,128]
        "pb": bf(p["pool_b"].reshape(1, H)),
        "identf": np.eye(128, dtype=np.float32),
        "identb": np.eye(128).astype(ml_dtypes.bfloat16),
    }

    in_maps = []
    for c in range(N_CORES):
        el, eh = c * ES, (c + 1) * ES
        nl, nh = c * NS, (c + 1) * NS
        ml, mh = c * MS, (c + 1) * MS
        us_s, vs_s = us[el:eh], vs[el:eh]
        nfTs = np.concatenate([nf[nl:nh].T, np.ones((1, NS), np.float32)], 0)
        efTs = np.concatenate([ef[el:eh].T, np.ones((1, ES), np.float32)], 0)
        m = dict(common)
        m.update({
            "nfTs": bf(nfTs),                          # [65, 512]
            "efTs": bf(efTs),                          # [17, 1024]
            "idxall": np.ascontiguousarray(np.concatenate(
                [wrap16(us_s.astype(np.int16)),
                 wrap16(vs_s.astype(np.int16))], axis=1)),
            "mnTs": np.ascontiguousarray(mn[ml:mh, nl:nh].T),  # [512, 32]
            # incidence shard transposed, as [128, ET, N] bf16 (0/1 exact)
            "nesT": bf(np.ascontiguousarray(
                ne[:, el:eh].T.reshape(ET, 128, N).transpose(1, 0, 2))),
        })
        in_maps.append(m)
    return in_maps


def assemble_outputs(results):
    readout = np.concatenate([results[c]["ro"] for c in range(N_CORES)], 0)
    wts = np.zeros((M, N), np.float32)
    for c in range(N_CORES):
        wts[c * MS:(c + 1) * MS, c * NS:(c + 1) * NS] = results[c]["wts"]
    return readout, wts


def build_kernel(plan=None, reps=1):
    nc = bacc.Bacc("TRN2", target_bir_lowering=False, debug=False,
                   num_devices=N_CORES)

    din = {}

    def inp(name, shape, dt):
        din[name] = nc.dram_tensor(name, list(shape), dt, kind="ExternalInput")
        return din[name]

    inp("nfpad", (N, 128), BF16)
    inp("nfTs", (ND + 1, NS), BF16)
    inp("efTs", (ED + 1, ES), BF16)
    inp("fcnwT", (ND + 1, H), BF16)
    inp("fcewT", (ED + 1, HE), BF16)
    inp("mwu", (H, L * H), BF16); inp("mwv", (H, L * H), BF16)
    inp("mwe", (HE + 1, L * H), BF16)
    inp("mewu", (H, L * HE), BF16); inp("mewv", (H, L * HE), BF16)
    inp("mewe", (HE + 1, L * HE), BF16)
    inp("maw", (1, L * H), F32); inp("mab", (1, L), F32)
    inp("gwih", (H, L * 3 * H), BF16); inp("gwhh", (H, L * 3 * H), BF16)
    inp("gbh", (1, L * 3 * H), BF16)
    inp("gbin", (1, L * H), F32)
    inp("paw", (H, 1), BF16); inp("pab", (1, 1), F32)
    inp("pw", (H, H), BF16); inp("pb", (1, H), BF16)
    inp("mnTs", (NS, MS), F32)
    inp("identf", (128, 128), F32)
    inp("identb", (128, 128), BF16)
    inp("idxall", (128, 2 * (ES // 16)), I16)
    inp("nesT", (128, ET, N), BF16)

    out_ro = nc.dram_tensor("ro", [MS, H], F32, kind="ExternalOutput")
    out_wts = nc.dram_tensor("wts", [MS, NS], F32, kind="ExternalOutput")

    acc = nc.dram_tensor("acc", [N_CORES * AC_R, AC_C], F32, kind="Internal")
    acc_r = [nc.dram_tensor(f"accr{i}", [AC_R, AC_C], F32, kind="Internal")
             for i in range(L)]
    tables = [nc.dram_tensor(f"table{i}", [N, H], BF16, kind="Internal",
                             addr_space="Shared") for i in (1, 2)]
    agin = [nc.dram_tensor(f"agin{i}", [NS, H], BF16, kind="Internal")
            for i in (1, 2)]

    with tile.TileContext(nc) as tc:
        with tc.tile_pool(name="const", bufs=1) as cpool, \
             tc.tile_pool(name="wts_sb", bufs=1) as wpool, \
             tc.tile_pool(name="big", bufs=1) as bigpool, \
             tc.tile_pool(name="work", bufs=2) as work, \
             tc.tile_pool(name="hid", bufs=2) as hidp, \
             tc.tile_pool(name="hnp", bufs=2) as hnpp, \
             tc.tile_pool(name="small", bufs=4) as small, \
             tc.tile_pool(name="ps_mm", bufs=2, space="PSUM") as ps_mm, \
             tc.tile_pool(name="ps_mmb", bufs=1, space="PSUM") as ps_mmb, \
             tc.tile_pool(name="ps_agg", bufs=1, space="PSUM") as ps_agg, \
             tc.tile_pool(name="ps_e", bufs=1, space="PSUM") as ps_e, \
             tc.tile_pool(name="ps_s", bufs=1, space="PSUM") as ps_s, \
             tc.tile_pool(name="ps_g", bufs=1, space="PSUM") as ps_g:

            def lrelu(psum_ap, out_ap, shape):
                tmp = work.tile(shape, F32, tag=f"lrt{shape[0]}x{shape[1]}")
                nc.scalar.activation(tmp[:], psum_ap, AF.Copy, scale=0.01)
                nc.vector.tensor_max(out_ap, psum_ap, tmp[:])

            ones_col = cpool.tile([128, 1], F32)
            nc.vector.memset(ones_col[:], 1.0)
            ones_colb = cpool.tile([128, 1], BF16)
            nc.vector.memset(ones_colb[:], 1.0)
            ones_row = cpool.tile([1, 128], BF16)
            nc.vector.memset(ones_row[:], 1.0)
            ones_rowf = cpool.tile([1, 128], F32)
            nc.vector.memset(ones_rowf[:], 1.0)

            def load(name, shape, dt):
                t = wpool.tile(shape, dt, tag=name)
                nc.sync.dma_start(t[:], din[name][:])
                return t

            idxall = load("idxall", [128, 2 * (ES // 16)], I16)
            iw = ES // 16
            idx = {"usg": idxall[:, 0:iw], "vsg": idxall[:, iw:2 * iw]}
            fcnwT = load("fcnwT", [ND + 1, H], BF16)
            ident = load("identf", [128, 128], F32)
            identb = load("identb", [128, 128], BF16)
            fcewT = load("fcewT", [ED + 1, HE], BF16)
            nfTs = load("nfTs", [ND + 1, NS], BF16)
            efTs = load("efTs", [ED + 1, ES], BF16)
            nesT = load("nesT", [128, ET, N], BF16)
            W = {}
            for nm, r, cdim in (
                ("mwu", H, H), ("mwv", H, H), ("mwe", HE + 1, H),
                ("mewu", H, HE), ("mewv", H, HE), ("mewe", HE + 1, HE),
                ("gwih", H, 3 * H), ("gwhh", H, 3 * H),
            ):
                W[nm] = (load(nm, [r, L * cdim], BF16), cdim)
            gbh = load("gbh", [1, L * 3 * H], BF16)
            gbin = load("gbin", [1, L * H], F32)
            maw_r = load("maw", [1, L * H], F32)
            mab_r = load("mab", [1, L], F32)
            pab_r = load("pab", [1, 1], F32)
            paw = load("paw", [H, 1], BF16)
            pw = load("pw", [H, H], BF16)
            pb = load("pb", [1, H], BF16)
            mnTs = wpool.tile([128, NT, MS], F32, tag="mnTs")
            for t in range(NT):
                nc.sync.dma_start(mnTs[:, t, :], din["mnTs"][ts(t, 128), :])

            def bcast128(dst_ap, src_row_ap, width, nparts=128):
                pbc = ps_mm.tile([128, 512], F32, tag="mm")
                nc.tensor.matmul(pbc[:nparts, 0:width],
                                 ones_rowf[:, 0:nparts], src_row_ap,
                                 start=True, stop=True)
                nc.scalar.activation(dst_ap, pbc[:nparts, 0:width], AF.Copy)

            mawB = cpool.tile([128, L * H], F32)
            bcast128(mawB[:], maw_r[:], L * H)
            mabB = cpool.tile([128, L], F32)
            bcast128(mabB[:], mab_r[:], L)
            pabB = cpool.tile([128, 1], F32)
            bcast128(pabB[:], pab_r[:], 1)
            gbinB = cpool.tile([128, L * H], F32)
            bcast128(gbinB[:], gbin[:], L * H)

            def Wl(nm, i):
                t, cdim = W[nm]
                return t[:, ts(i, cdim)]

            for rep in range(reps):
                # ---------- h0 on own node stripe: hT [128, 512] bf16 -----
                hT = bigpool.tile([H, NS], BF16, tag=f"hT_{rep % 2}")
                ps0 = ps_mm.tile([128, NS], F32, tag="mm")
                nc.tensor.matmul(ps0[:], fcnwT[:], nfTs[:], start=True,
                                 stop=True)
                lrelu(ps0[:], hT[:], [H, NS])
                h_np = []
                for t in range(NT):
                    pst = ps_mmb.tile([128, 128], BF16, tag="mmb")
                    nc.tensor.transpose(pst[:], hT[:, ts(t, 128)], identb[:])
                    ht = hnpp.tile([128, 128], F32, tag=f"hnp{t}")
                    nc.scalar.activation(ht[:], pst[:], AF.Copy)
                    h_np.append(ht)

                # ---------- e0: eT [65, 1024] bf16 ------------------------
                eT = bigpool.tile([HE + 1, ES], BF16, tag=f"eT_{rep % 2}")
                nc.vector.memset(eT[HE:HE + 1, :], 1.0)
                for ch in range(2):
                    pse = ps_e.tile([HE, 512], F32, tag="e")
                    nc.tensor.matmul(pse[:], fcewT[:], efTs[:, ts(ch, 512)],
                                     start=True, stop=True)
                    lrelu(pse[:], eT[0:HE, ts(ch, 512)], [HE, 512])

                huT = bigpool.tile([128, ES], BF16, tag="huT")
                hvT = bigpool.tile([128, ES], BF16, tag="hvT")
                gu = bigpool.tile([128, ET, 128], BF16, tag="gu")
                gv = bigpool.tile([128, ET, 128], BF16, tag="gv")

                for i in range(L):
                    # ---- gather endpoint rows (edge-partition tiles) ----
                    src_tbl = din["nfpad"] if i == 0 else tables[i - 1]
                    nc.gpsimd.dma_gather(gu[:], src_tbl[:], idx["usg"],
                                         ES, ES, 128, transpose=False,
                                         single_packet=False)
                    nc.gpsimd.dma_gather(gv[:], src_tbl[:], idx["vsg"],
                                         ES, ES, 128, transpose=False,
                                         single_packet=False)
                    if i == 0:
                        for gsrc, gdstT, gtag in ((gu, huT, "gTu"),
                                                  (gv, hvT, "gTv")):
                            gT = bigpool.tile([128, ES], BF16, tag=gtag)
                            for t in range(ET):
                                pst = ps_mmb.tile([128, 128], BF16, tag="mmb")
                                nc.tensor.transpose(pst[:], gsrc[:, t:t + 1, :],
                                                    identb[:])
                                nc.scalar.activation(gT[:, ts(t, 128)],
                                                     pst[:], AF.Copy)
                            for ch in range(2):
                                psh = ps_mm.tile([128, 512], F32, tag="mm")
                                nc.tensor.matmul(
                                    psh[:], fcnwT[:],
                                    gT[0:ND + 1, ts(ch, 512)],
                                    start=True, stop=True)
                                lrelu(psh[:], gdstT[:, ts(ch, 512)],
                                      [128, 512])
                    else:
                        for gsrc, gdstT in ((gu, huT), (gv, hvT)):
                            for t in range(ET):
                                pst = ps_mmb.tile([128, 128], BF16, tag="mmb")
                                nc.tensor.transpose(pst[:], gsrc[:, t:t + 1, :],
                                                    identb[:])
                                nc.scalar.activation(gdstT[:, ts(t, 128)],
                                                     pst[:], AF.Copy)

                    # ---- e' (skip on last layer: unused) ----
                    new_eT = None
                    if i < L - 1:
                        new_eT = bigpool.tile([HE + 1, ES], BF16,
                                              tag=f"eT_{(rep + i + 1) % 2}")
                        nc.vector.memset(new_eT[HE:HE + 1, :], 1.0)
                        for ch in range(2):
                            pse = ps_e.tile([HE, 512], F32, tag="e")
                            nc.tensor.matmul(pse[:], Wl("mewu", i),
                                             huT[:, ts(ch, 512)],
                                             start=True, stop=False)
                            nc.tensor.matmul(pse[:], Wl("mewv", i),
                                             hvT[:, ts(ch, 512)],
                                             start=False, stop=False)
                            nc.tensor.matmul(pse[:], Wl("mewe", i),
                                             eT[:, ts(ch, 512)],
                                             start=False, stop=True)
                            lrelu(pse[:], new_eT[0:HE, ts(ch, 512)],
                                  [HE, 512])

                    # ---- hid per e-tile -> x = p*hid (bf16) + S ----
                    xs = []
                    ps_ = []
                    psS = ps_s.tile([128, 1], F32, tag="s")
                    for t in range(ET):
                        psh = ps_mm.tile([128, 128], F32, tag="mm")
                        nc.tensor.matmul(psh[:], huT[:, ts(t, 128)],
                                         Wl("mwu", i), start=True, stop=False)
                        nc.tensor.matmul(psh[:], hvT[:, ts(t, 128)],
                                         Wl("mwv", i), start=False, stop=False)
                        nc.tensor.matmul(psh[:], eT[:, ts(t, 128)],
                                         Wl("mwe", i), start=False, stop=True)
                        hid_t = hidp.tile([128, 128], F32, tag=f"hid{t % 2}")
                        lrelu(psh[:], hid_t[:], [128, 128])
                        # S^T column: accumulate sum over edges of hid
                        nc.tensor.matmul(psS[:], hid_t[:], ones_col[:],
                                         start=(t == 0), stop=(t == ET - 1))
                        am = work.tile([128, 128], F32, tag="am")
                        nc.vector.tensor_mul(am[:], hid_t[:],
                                             mawB[:, ts(i, H)])
                        a_t = small.tile([128, 1], F32, tag="a_t")
                        nc.vector.reduce_sum(a_t[:], am[:],
                                             axis=mybir.AxisListType.X)
                        p_f = small.tile([128, 1], F32, tag="p_f")
                        nc.scalar.activation(p_f[:], a_t[:], AF.Exp,
                                             bias=mabB[:, i:i + 1])
                        p_t = small.tile([128, 1], BF16, tag=f"p_t{t}")
                        nc.scalar.activation(p_t[:], p_f[:], AF.Copy)
                        x_t = hidp.tile([128, 128], BF16, tag=f"x{t}")
                        nc.scalar.activation(x_t[:], hid_t[:], AF.Copy,
                                             scale=p_f[:])
                        xs.append(x_t)
                        ps_.append(p_t)

                    # ---- aggregation matmuls against the incidence shard:
                    # chunk ch of 512 nodes == stripe block of core ch ----
                    sT = small.tile([128, 1], F32, tag="sT")
                    nc.scalar.activation(sT[:], psS[:], AF.Copy)
                    for ch in range(N // 512):
                        psa = ps_agg.tile([128, 512], F32, tag="agg")
                        psz = ps_s.tile([1, 512], F32, tag="s")
                        for t in range(ET):
                            nc.tensor.matmul(psa[:], xs[t][:],
                                             nesT[:, t, ts(ch, 512)],
                                             start=(t == 0), stop=(t == ET - 1))
                            nc.tensor.matmul(psz[:], ps_[t][:],
                                             nesT[:, t, ts(ch, 512)],
                                             start=(t == 0), stop=(t == ET - 1))
                        cst = work.tile([128, 512], F32, tag="cst")
                        nc.scalar.activation(cst[:], psa[:], AF.Copy)
                        zst = small.tile([1, 512], F32, tag="zst")
                        nc.vector.tensor_copy(zst[:], psz[:])
                        base = ch * AC_R
                        nc.sync.dma_start(acc[base:base + 128, 0:NS], cst[:])
                        nc.sync.dma_start(acc[base + 128:base + 129, 0:NS],
                                          zst[:])
                        nc.sync.dma_start(acc[base:base + 128, NS:NS + 1],
                                          sT[:])
                    nc.gpsimd.collective_compute(
                        "ReduceScatter", ALU.add, replica_groups=RG,
                        ins=[acc[:]], outs=[acc_r[i][:]])

                    # ---- ctx stripe + GRU per node tile ----
                    ctxf = work.tile([128, NS], F32, tag="ctxfs")
                    nc.sync.dma_start(ctxf[:], acc_r[i][0:128, 0:NS])
                    ctxT = work.tile([128, NS], BF16, tag="ctxTs")
                    nc.scalar.activation(ctxT[:], ctxf[:], AF.Copy)
                    zcol = small.tile([128, NT], F32, tag="zcol")
                    nc.sync.dma_start(
                        zcol[:],
                        acc_r[i][128:129, 0:NS].rearrange(
                            "o (t p) -> (o p) t", p=128))
                    scol = small.tile([128, 1], F32, tag="scol")
                    nc.sync.dma_start(scol[:], acc_r[i][0:128, NS:NS + 1])
                    ssc = small.tile([128, 1], BF16, tag="ssc")
                    nc.scalar.activation(ssc[:], scol[:], AF.Copy,
                                         scale=1.0 / E)
                    # SW = (S/E) @ wih.T  [1, 384]
                    psw = ps_g.tile([128, 3 * H], F32, tag="psgi")
                    nc.tensor.matmul(psw[0:1, :], ssc[:], Wl("gwih", i),
                                     start=True, stop=True)
                    SW = small.tile([1, 3 * H], F32, tag="SW")
                    nc.vector.tensor_copy(SW[:], psw[0:1, :])
                    SWb = work.tile([128, 3 * H], F32, tag="SWb")
                    bcast128(SWb[:], SW[:], 3 * H)

                    hT_new = bigpool.tile([H, NS], BF16,
                                          tag=f"hT_{(rep + i + 1) % 2}")
                    hbf_all = work.tile([128, NT, 128], BF16, tag="hbf_all")
                    h_np_new = []
                    for t in range(NT):
                        Zs = small.tile([128, 1], F32, tag="Zs")
                        nc.vector.tensor_scalar_max(Zs[:], zcol[:, t:t + 1],
                                                    1e-37)
                        rz = small.tile([128, 1], F32, tag="rz")
                        nc.vector.reciprocal(rz[:], Zs[:])
                        mk = small.tile([128, 1], F32, tag="mk")
                        nc.vector.tensor_scalar(mk[:], zcol[:, t:t + 1], 0.0,
                                                None, op0=ALU.is_equal)
                        psgi = ps_g.tile([128, 3 * H], F32, tag="psgi")
                        nc.tensor.matmul(psgi[:], ctxT[:, ts(t, 128)],
                                         Wl("gwih", i), start=True, stop=True)
                        gi = work.tile([128, 3 * H], F32, tag="gi")
                        nc.scalar.activation(gi[:], psgi[:], AF.Copy,
                                             scale=rz[:])
                        iso = work.tile([128, 3 * H], F32, tag="iso")
                        nc.vector.tensor_scalar_mul(iso[:], SWb[:], mk[:])
                        nc.vector.tensor_add(gi[:], gi[:], iso[:])
                        psgh = ps_g.tile([128, 3 * H], F32, tag="psgh")
                        nc.tensor.matmul(psgh[:], hT[:, ts(t, 128)],
                                         Wl("gwhh", i), start=True, stop=False)
                        nc.tensor.matmul(psgh[:], ones_row[:],
                                         gbh[:, ts(i, 3 * H)],
                                         start=False, stop=True)
                        rs_ = work.tile([128, 128], F32, tag="rsum")
                        nc.vector.tensor_add(rs_[:], psgh[:, 0:128],
                                             gi[:, 0:128])
                        r_t = work.tile([128, 128], F32, tag="r_t")
                        nc.scalar.activation(r_t[:], rs_[:], AF.Sigmoid)
                        zs_ = work.tile([128, 128], F32, tag="zsum")
                        nc.vector.tensor_add(zs_[:], psgh[:, 128:256],
                                             gi[:, 128:256])
                        z_t = work.tile([128, 128], F32, tag="z_t")
                        nc.scalar.activation(z_t[:], zs_[:], AF.Sigmoid)
                        rhn = work.tile([128, 128], F32, tag="rhn")
                        nc.vector.tensor_mul(rhn[:], r_t[:], psgh[:, 256:384])
                        nc.vector.tensor_add(rhn[:], rhn[:], gi[:, 256:384])
                        nc.vector.tensor_add(rhn[:], rhn[:],
                                             gbinB[:, ts(i, H)])
                        n_t = work.tile([128, 128], F32, tag="n_t")
                        nc.scalar.activation(n_t[:], rhn[:], AF.Tanh)
                        hmn = work.tile([128, 128], F32, tag="hmn")
                        nc.vector.tensor_sub(hmn[:], h_np[t][:], n_t[:])
                        nc.vector.tensor_mul(hmn[:], z_t[:], hmn[:])
                        h2 = hnpp.tile([128, 128], F32, tag=f"hnp{t}")
                        nc.vector.tensor_add(h2[:], n_t[:], hmn[:])
                        if i < L - 1:
                            nc.scalar.activation(h2[:], h2[:], AF.Relu)
                            nc.scalar.activation(hbf_all[:, t, :], h2[:],
                                                 AF.Copy)
                        pst2 = ps_mm.tile([128, 128], F32, tag="mm")
                        nc.tensor.transpose(pst2[:], h2[:], ident[:])
                        nc.scalar.activation(hT_new[:, ts(t, 128)], pst2[:],
                                             AF.Copy)
                        h_np_new.append(h2)

                    if i < L - 1:
                        nc.sync.dma_start(
                            agin[i][:].rearrange("(t p) f -> p t f", p=128),
                            hbf_all[:])
                        nc.gpsimd.collective_compute(
                            "AllGather", ALU.bypass, replica_groups=RG,
                            ins=[agin[i][:]], outs=[tables[i][:]])
                        eT = new_eT
                    hT = hT_new
                    h_np = h_np_new

                # ---------------- pooling ----------------
                ps_at = ps_s.tile([128, NT], F32, tag="s")
                expat = []
                for t in range(NT):
                    nc.tensor.matmul(ps_at[:, t:t + 1], hT[:, ts(t, 128)],
                                     paw[:], start=True, stop=True)
                    ea = small.tile([128, 1], F32, tag=f"expat{t}")
                    nc.scalar.activation(ea[:], ps_at[:, t:t + 1], AF.Exp,
                                         bias=pabB[:])
                    expat.append(ea)
                wtsu = []
                psZ = ps_e.tile([1, MS], F32, tag="e")
                for t in range(NT):
                    wu = work.tile([128, MS], F32, tag=f"wtsu{t}")
                    nc.scalar.activation(wu[:], mnTs[:, t, :], AF.Copy,
                                         scale=expat[t][:])
                    nc.tensor.matmul(psZ[:], ones_col[:], wu[:],
                                     start=(t == 0), stop=(t == NT - 1))
                    wtsu.append(wu)
                rZ = small.tile([1, MS], F32, tag="rZ")
                nc.vector.reciprocal(rZ[:], psZ[:])
                rZb = work.tile([128, MS], F32, tag="rZb")
                bcast128(rZb[:], rZ[:], MS)

                ps_ro = ps_g.tile([MS, H], F32, tag="psgi")
                wband = bigpool.tile([MS, NS], F32, tag="wband")
                for t in range(NT):
                    wf = work.tile([128, MS], F32, tag="wf")
                    nc.vector.tensor_mul(wf[:], wtsu[t][:], rZb[:])
                    psg = ps_mm.tile([128, H], F32, tag="mm")
                    nc.tensor.matmul(psg[:], hT[:, ts(t, 128)], pw[:],
                                     start=True, stop=False)
                    nc.tensor.matmul(psg[:], ones_row[:], pb[:],
                                     start=False, stop=True)
                    g_t = work.tile([128, H], F32, tag="g_t")
                    lrelu(psg[:], g_t[:], [128, H])
                    nc.tensor.matmul(ps_ro[:], wf[:], g_t[:],
                                     start=(t == 0), stop=(t == NT - 1))
                    pswt = ps_g.tile([MS, 128], F32, tag="psgh")
                    nc.tensor.transpose(pswt[:], wf[:], ident[:])
                    nc.scalar.activation(wband[:, ts(t, 128)], pswt[:],
                                         AF.Copy)

                ro_sb = work.tile([MS, H], F32, tag="ro_sb")
                nc.scalar.activation(ro_sb[:], ps_ro[:], AF.Copy)
                nc.sync.dma_start(out_ro[:], ro_sb[:])
                nc.sync.dma_start(out_wts[:], wband[:])

    nc.compile()
    return nc


# ----------------------------------------------------------------------------
# PJRT SPMD runner (inlined; no NTFF profiling available under this axon
# deployment).  The neuron NEFF cache fingerprints the HLO without the
# custom-call backend_config (where the BIR lives), so the jitted function
# carries an unused parameter whose shape encodes a hash of the BIR.
# ----------------------------------------------------------------------------
import hashlib
import jax
from jax.sharding import Mesh, PartitionSpec
from jax.experimental.shard_map import shard_map
from concourse import bass2jax


class _SpmdRunner:
    def __init__(self, nc, n_cores):
        bass2jax.install_neuronx_cc_hook()
        self.nc = nc
        self.n_cores = n_cores
        partition_name = (
            nc.partition_id_tensor.name if nc.partition_id_tensor else None
        )
        in_names, out_names, out_avals = [], [], []
        for alloc in nc.m.functions[0].allocations:
            if not isinstance(alloc, mybir.MemoryLocationSet):
                continue
            name = alloc.memorylocations[0].name
            if alloc.kind == "ExternalInput":
                if name != partition_name:
                    in_names.append(name)
            elif alloc.kind == "ExternalOutput":
                out_names.append(name)
                out_avals.append(jax.core.ShapedArray(
                    tuple(alloc.tensor_shape), mybir.dt.np(alloc.dtype)))
        self.in_names, self.out_names, self.out_avals = \
            in_names, out_names, out_avals
        n_params = len(in_names)
        all_in_names = list(in_names) + list(out_names)
        if partition_name is not None:
            all_in_names.append(partition_name)
        self._zero_outs = [
            np.zeros((n_cores * a.shape[0], *a.shape[1:]), a.dtype)
            for a in out_avals]
        bir_hash = int(hashlib.sha256(nc.to_json_bytes()).hexdigest()[:8], 16)
        self._salt = np.zeros((1 + bir_hash % 509,
                               1 + (bir_hash // 509) % 127), np.int8)

        def _body(*args):
            operands = list(args[:-1])
            if partition_name is not None:
                operands.append(bass2jax.partition_id_tensor())
            return tuple(bass2jax._bass_exec_p.bind(
                *operands,
                out_avals=tuple(out_avals),
                in_names=tuple(all_in_names),
                out_names=tuple(out_names),
                lowering_input_output_aliases=(),
                sim_require_finite=False,
                sim_require_nnan=False,
                nc=nc,
            ))

        devices = jax.devices()[:n_cores]
        mesh = Mesh(np.asarray(devices), ("core",))
        in_specs = (PartitionSpec("core"),) * (n_params + len(out_names)) + (
            PartitionSpec(),)
        out_specs = (PartitionSpec("core"),) * len(out_names)
        self._fn = jax.jit(
            shard_map(_body, mesh=mesh, in_specs=in_specs,
                      out_specs=out_specs, check_rep=False),
            keep_unused=True)

    def run(self, in_maps):
        n = self.n_cores
        concat = [
            np.concatenate([np.asarray(in_maps[c][k]) for c in range(n)], 0)
            for k in self.in_names]
        args = concat + self._zero_outs + [self._salt]
        outs = self._fn(*args)
        outs = [np.asarray(o) for o in outs]
        return [
            {name: outs[j].reshape(n, *self.out_avals[j].shape)[c]
             for j, name in enumerate(self.out_names)}
            for c in range(n)]


_CACHED = {}


def kernel(**inputs):
    """Full (unsharded) inputs -> full outputs (readout [256,128],
    wts [256,4096]), computed on 8 TRN2 NeuronCores."""
    if "nc" not in _CACHED:
        _CACHED["nc"] = build_kernel(reps=1)
        _CACHED["runner"] = _SpmdRunner(_CACHED["nc"], N_CORES)
    in_maps = prep_inputs(inputs)
    results = _CACHED["runner"].run(in_maps)
    return assemble_outputs(results)


# revision 2
# speedup vs baseline: 1.5771x; 1.5771x over previous
"""AMPNN (gnn_message_passing) distributed Bass kernel for 8 TRN2 cores.

Strategy (matches the problem's sharding hint): column-shard the
[n_node, n_edge] incidence matrix over edges, run the masked softmax +
aggregation as TensorE matmuls against the bf16 incidence shard held in
SBUF, and AllReduce the per-node context.

node_edge_mask = where(ne>0, 0, -1e6) makes the reference softmax an exact
per-node softmax over incident edges (non-incident terms underflow to 0),
so per edge we compute p = exp(a) and accumulate
  ctx_raw[:, n] = sum_e (p*hid)[e, :] * ne[n, e]      (128 x n matmul)
  Z[n]          = sum_e p[e] * ne[n, e]               (1 x n matmul)
with ne_shard^T [1024, 4096] as the moving operand (bf16, exact for 0/1).

- edges sharded 1024/core; nodes 512/core; mols 32/core
- gather h[us], h[vs] via gpsimd.dma_gather from a bf16 node table in DRAM
  (layer 0 gathers padded node_features instead and applies the input MLP
  per-edge: lrelu/gather commute); PE transposes to channel-partition
- per-edge MLPs as TensorE matmuls with channels on partitions (bf16)
- the accumulator is laid out as 8 row-blocks of [129, 513] (ctx^T
  channels + Z row, one block per node stripe, + an S column for the
  isolated-node uniform-softmax fallback), so ONE ReduceScatter per layer
  hands each core exactly its stripe with no core-dependent addressing
- the ReduceScattered ctx^T block is directly the GRU's lhsT (no
  transposes);
  normalization by 1/Z and the isolated-node fallback are applied to the
  matmul output with per-partition scales / rank-1 terms
- GRU on the local node stripe (bf16 matmuls, f32 gates/state); relu'd h
  AllGathered in bf16 as the next layer's gather table
- attentive pooling per mol stripe (16 contiguous nodes/mol); host embeds
  the [32, 512] wts band into the exact-zero [256, 4096] output
"""
import os
import numpy as np
import ml_dtypes
import concourse.bass as bass
import concourse.bacc as bacc
import concourse.tile as tile
import concourse.mybir as mybir

F32 = mybir.dt.float32
BF16 = mybir.dt.bfloat16
I16 = mybir.dt.int16
AF = mybir.ActivationFunctionType
ALU = mybir.AluOpType
ts = bass.ts

N_CORES = 8
N, E, M = 4096, 8192, 256
ND, ED, H, HE, C, L = 64, 16, 128, 64, 128, 3
ES = E // N_CORES          # 1024 edges/core
NS = N // N_CORES          # 512 nodes/core
MS = M // N_CORES          # 32 mols/core
ET = ES // 128             # 8 edge tiles
NT = NS // 128             # 4 node tiles
AC_R = 129                 # block rows: 128 ctx channels + Z row
AC_C = NS + 1              # block cols: 512 stripe nodes + S column
RG = [list(range(N_CORES))]


def wrap16(a):
    """[n] -> [128, n//16]: 16-partition wrap (unwrapped[i] ==
    wrapped[i % 16, i // 16]) replicated to all 128 partitions (the 8
    gpsimd Q7 cores each read their own 16-partition copy)."""
    w = np.ascontiguousarray(a.reshape(-1, 16).T)
    return np.tile(w, (8, 1))


def bf(x):
    return np.ascontiguousarray(x).astype(ml_dtypes.bfloat16)


def prep_inputs(inputs):
    """Full problem inputs -> per-core in_maps (host-side shard/layout)."""
    nf = np.asarray(inputs["node_features"], np.float32)
    ef = np.asarray(inputs["edge_features"], np.float32)
    us = np.asarray(inputs["us"], np.int64)
    vs = np.asarray(inputs["vs"], np.int64)
    mn = np.asarray(inputs["mol_node_matrix"], np.float32)
    ne = np.asarray(inputs["node_edge_matrix"], np.float32)
    p = {k: np.asarray(inputs[k], np.float32) for k in (
        "fcn_w", "fcn_b", "fce_w", "fce_b", "m_w", "m_b", "me_w", "me_b",
        "ma_w", "ma_b", "g_wih", "g_whh", "g_bih", "g_bhh",
        "pool_at_w", "pool_at_b", "pool_w", "pool_b")}

    nfpad = np.zeros((N, 128), np.float32)
    nfpad[:, :ND] = nf
    nfpad[:, ND] = 1.0

    fcnwT = np.concatenate([p["fcn_w"].T, p["fcn_b"][None, :]], 0)   # [65,128]
    fcewT = np.concatenate([p["fce_w"].T, p["fce_b"][None, :]], 0)   # [17,64]
    hs = np.hstack
    mwu = hs([p["m_w"][i][:, 0:H].T for i in range(L)])              # [128,3*128]
    mwv = hs([p["m_w"][i][:, H:2 * H].T for i in range(L)])
    mwe = hs([np.concatenate([p["m_w"][i][:, 2 * H:].T,
                              p["m_b"][i][None, :]], 0) for i in range(L)])
    mewu = hs([p["me_w"][i][:, 0:H].T for i in range(L)])            # [128,3*64]
    mewv = hs([p["me_w"][i][:, H:2 * H].T for i in range(L)])
    mewe = hs([np.concatenate([p["me_w"][i][:, 2 * H:].T,
                               p["me_b"][i][None, :]], 0) for i in range(L)])
    gwih = hs([p["g_wih"][i].T for i in range(L)])                   # [128,3*384]
    gwhh = hs([p["g_whh"][i].T for i in range(L)])
    # r,z bias slices can ride on the h-side rank-1 bias matmul; the n-slice
    # of g_bih must be added after the r*hn product, so it goes separately.
    gbh = np.concatenate([
        np.concatenate([p["g_bih"][i][0:2 * H] + p["g_bhh"][i][0:2 * H],
                        p["g_bhh"][i][2 * H:]]) for i in range(L)])
    gbin = hs([p["g_bih"][i][None, 2 * H:] for i in range(L)])       # [1,3*128]

    common = {
        "nfpad": bf(nfpad),
        "fcnwT": bf(fcnwT), "fcewT": bf(fcewT),
        "mwu": bf(mwu), "mwv": bf(mwv), "mwe": bf(mwe),
        "mewu": bf(mewu), "mewv": bf(mewv), "mewe": bf(mewe),
        "maw": np.ascontiguousarray(p["ma_w"].reshape(1, L * C)),
        "mab": np.ascontiguousarray(p["ma_b"].reshape(1, L)),
        "gwih": bf(gwih), "gwhh": bf(gwhh),
        "gbh": bf(gbh.reshape(1, L * 3 * H)),
        "gbin": np.ascontiguousarray(gbin),                          # f32
        "paw": bf(p["pool_at_w"].T),                                 # [128,1]
        "pab": p["pool_at_b"].reshape(1, 1).astype(np.float32),
        "pw": bf(p["pool_w"].T),                                     # [128,128]
        "pb": bf(p["pool_b"].reshape(1, H)),
        "identf": np.eye(128, dtype=np.float32),
        "identb": np.eye(128).astype(ml_dtypes.bfloat16),
    }

    in_maps = []
    for c in range(N_CORES):
        el, eh = c * ES, (c + 1) * ES
        nl, nh = c * NS, (c + 1) * NS
        ml, mh = c * MS, (c + 1) * MS
        us_s, vs_s = us[el:eh], vs[el:eh]
        nfTs = np.concatenate([nf[nl:nh].T, np.ones((1, NS), np.float32)], 0)
        efTs = np.concatenate([ef[el:eh].T, np.ones((1, ES), np.float32)], 0)
        m = dict(common)
        m.update({
            "nfTs": bf(nfTs),                          # [65, 512]
            "efTs": bf(efTs),                          # [17, 1024]
            "idxall": np.ascontiguousarray(np.concatenate(
                [wrap16(us_s.astype(np.int16)),
                 wrap16(vs_s.astype(np.int16))], axis=1)),
            "mnTs": np.ascontiguousarray(mn[ml:mh, nl:nh].T),  # [512, 32]
            # incidence shard transposed, as [128, ET, N] bf16 (0/1 exact)
            "nesT": bf(np.ascontiguousarray(
                ne[:, el:eh].T.reshape(ET, 128, N).transpose(1, 0, 2))),
        })
        in_maps.append(m)
    return in_maps


def assemble_outputs(results):
    readout = np.concatenate([results[c]["ro"] for c in range(N_CORES)], 0)
    wts = np.zeros((M, N), np.float32)
    for c in range(N_CORES):
        wts[c * MS:(c + 1) * MS, c * NS:(c + 1) * NS] = results[c]["wts"]
    return readout, wts


def build_kernel(plan=None, reps=1):
    nc = bacc.Bacc("TRN2", target_bir_lowering=False, debug=False,
                   num_devices=N_CORES)

    din = {}

    def inp(name, shape, dt):
        din[name] = nc.dram_tensor(name, list(shape), dt, kind="ExternalInput")
        return din[name]

    inp("nfpad", (N, 128), BF16)
    inp("nfTs", (ND + 1, NS), BF16)
    inp("efTs", (ED + 1, ES), BF16)
    inp("fcnwT", (ND + 1, H), BF16)
    inp("fcewT", (ED + 1, HE), BF16)
    inp("mwu", (H, L * H), BF16); inp("mwv", (H, L * H), BF16)
    inp("mwe", (HE + 1, L * H), BF16)
    inp("mewu", (H, L * HE), BF16); inp("mewv", (H, L * HE), BF16)
    inp("mewe", (HE + 1, L * HE), BF16)
    inp("maw", (1, L * H), F32); inp("mab", (1, L), F32)
    inp("gwih", (H, L * 3 * H), BF16); inp("gwhh", (H, L * 3 * H), BF16)
    inp("gbh", (1, L * 3 * H), BF16)
    inp("gbin", (1, L * H), F32)
    inp("paw", (H, 1), BF16); inp("pab", (1, 1), F32)
    inp("pw", (H, H), BF16); inp("pb", (1, H), BF16)
    inp("mnTs", (NS, MS), F32)
    inp("identf", (128, 128), F32)
    inp("identb", (128, 128), BF16)
    inp("idxall", (128, 2 * (ES // 16)), I16)
    inp("nesT", (128, ET, N), BF16)

    out_ro = nc.dram_tensor("ro", [MS, H], F32, kind="ExternalOutput")
    out_wts = nc.dram_tensor("wts", [MS, NS], F32, kind="ExternalOutput")

    acc = nc.dram_tensor("acc", [N_CORES * AC_R, AC_C], BF16,
                         kind="Internal")
    acc_r = [nc.dram_tensor(f"accr{i}", [AC_R, AC_C], BF16, kind="Internal")
             for i in range(L)]
    tables = [nc.dram_tensor(f"table{i}", [N, H], BF16, kind="Internal",
                             addr_space="Shared") for i in (1, 2)]
    agin = [nc.dram_tensor(f"agin{i}", [NS, H], BF16, kind="Internal")
            for i in (1, 2)]

    with tile.TileContext(nc) as tc:
        with tc.tile_pool(name="const", bufs=1) as cpool, \
             tc.tile_pool(name="wts_sb", bufs=1) as wpool, \
             tc.tile_pool(name="big", bufs=1) as bigpool, \
             tc.tile_pool(name="work", bufs=2) as work, \
             tc.tile_pool(name="hid", bufs=2) as hidp, \
             tc.tile_pool(name="hnp", bufs=2) as hnpp, \
             tc.tile_pool(name="small", bufs=4) as small, \
             tc.tile_pool(name="ps_mm", bufs=2, space="PSUM") as ps_mm, \
             tc.tile_pool(name="ps_mmb", bufs=1, space="PSUM") as ps_mmb, \
             tc.tile_pool(name="ps_agg", bufs=1, space="PSUM") as ps_agg, \
             tc.tile_pool(name="ps_e", bufs=1, space="PSUM") as ps_e, \
             tc.tile_pool(name="ps_s", bufs=1, space="PSUM") as ps_s, \
             tc.tile_pool(name="ps_g", bufs=1, space="PSUM") as ps_g:

            def lrelu(psum_ap, out_ap, shape):
                tmp = work.tile(shape, F32, tag=f"lrt{shape[0]}x{shape[1]}")
                nc.scalar.activation(tmp[:], psum_ap, AF.Copy, scale=0.01)
                nc.vector.tensor_max(out_ap, psum_ap, tmp[:])

            ones_col = cpool.tile([128, 1], F32)
            nc.vector.memset(ones_col[:], 1.0)
            ones_colb = cpool.tile([128, 1], BF16)
            nc.vector.memset(ones_colb[:], 1.0)
            ones_row = cpool.tile([1, 128], BF16)
            nc.vector.memset(ones_row[:], 1.0)
            ones_rowf = cpool.tile([1, 128], F32)
            nc.vector.memset(ones_rowf[:], 1.0)

            def load(name, shape, dt):
                t = wpool.tile(shape, dt, tag=name)
                nc.sync.dma_start(t[:], din[name][:])
                return t

            idxall = load("idxall", [128, 2 * (ES // 16)], I16)
            iw = ES // 16
            idx = {"usg": idxall[:, 0:iw], "vsg": idxall[:, iw:2 * iw]}
            fcnwT = load("fcnwT", [ND + 1, H], BF16)
            ident = load("identf", [128, 128], F32)
            identb = load("identb", [128, 128], BF16)
            fcewT = load("fcewT", [ED + 1, HE], BF16)
            nfTs = load("nfTs", [ND + 1, NS], BF16)
            efTs = load("efTs", [ED + 1, ES], BF16)
            nesT = load("nesT", [128, ET, N], BF16)
            W = {}
            for nm, r, cdim in (
                ("mwu", H, H), ("mwv", H, H), ("mwe", HE + 1, H),
                ("mewu", H, HE), ("mewv", H, HE), ("mewe", HE + 1, HE),
                ("gwih", H, 3 * H), ("gwhh", H, 3 * H),
            ):
                W[nm] = (load(nm, [r, L * cdim], BF16), cdim)
            gbh = load("gbh", [1, L * 3 * H], BF16)
            gbin = load("gbin", [1, L * H], F32)
            maw_r = load("maw", [1, L * H], F32)
            mab_r = load("mab", [1, L], F32)
            pab_r = load("pab", [1, 1], F32)
            paw = load("paw", [H, 1], BF16)
            pw = load("pw", [H, H], BF16)
            pb = load("pb", [1, H], BF16)
            mnTs = wpool.tile([128, NT, MS], F32, tag="mnTs")
            for t in range(NT):
                nc.sync.dma_start(mnTs[:, t, :], din["mnTs"][ts(t, 128), :])

            def bcast128(dst_ap, src_row_ap, width, nparts=128):
                pbc = ps_mm.tile([128, 512], F32, tag="mm")
                nc.tensor.matmul(pbc[:nparts, 0:width],
                                 ones_rowf[:, 0:nparts], src_row_ap,
                                 start=True, stop=True)
                nc.scalar.activation(dst_ap, pbc[:nparts, 0:width], AF.Copy)

            mawB = cpool.tile([128, L * H], F32)
            bcast128(mawB[:], maw_r[:], L * H)
            mabB = cpool.tile([128, L], F32)
            bcast128(mabB[:], mab_r[:], L)
            pabB = cpool.tile([128, 1], F32)
            bcast128(pabB[:], pab_r[:], 1)
            gbinB = cpool.tile([128, L * H], F32)
            bcast128(gbinB[:], gbin[:], L * H)

            def Wl(nm, i):
                t, cdim = W[nm]
                return t[:, ts(i, cdim)]

            for rep in range(reps):
                # ---------- h0 on own node stripe: hT [128, 512] bf16 -----
                hT = bigpool.tile([H, NS], BF16, tag=f"hT_{rep % 2}")
                ps0 = ps_mm.tile([128, NS], F32, tag="mm")
                nc.tensor.matmul(ps0[:], fcnwT[:], nfTs[:], start=True,
                                 stop=True)
                lrelu(ps0[:], hT[:], [H, NS])
                h_np = []
                for t in range(NT):
                    pst = ps_mmb.tile([128, 128], BF16, tag="mmb")
                    nc.tensor.transpose(pst[:], hT[:, ts(t, 128)], identb[:])
                    ht = hnpp.tile([128, 128], F32, tag=f"hnp{t}")
                    nc.scalar.activation(ht[:], pst[:], AF.Copy)
                    h_np.append(ht)

                # ---------- e0: eT [65, 1024] bf16 ------------------------
                eT = bigpool.tile([HE + 1, ES], BF16, tag=f"eT_{rep % 2}")
                nc.vector.memset(eT[HE:HE + 1, :], 1.0)
                for ch in range(2):
                    pse = ps_e.tile([HE, 512], F32, tag="e")
                    nc.tensor.matmul(pse[:], fcewT[:], efTs[:, ts(ch, 512)],
                                     start=True, stop=True)
                    lrelu(pse[:], eT[0:HE, ts(ch, 512)], [HE, 512])

                huT = bigpool.tile([128, ES], BF16, tag="huT")
                hvT = bigpool.tile([128, ES], BF16, tag="hvT")
                gu = bigpool.tile([128, ET, 128], BF16, tag="gu")
                gv = bigpool.tile([128, ET, 128], BF16, tag="gv")

                for i in range(L):
                    # ---- gather endpoint rows (edge-partition tiles) ----
                    src_tbl = din["nfpad"] if i == 0 else tables[i - 1]
                    nc.gpsimd.dma_gather(gu[:], src_tbl[:], idx["usg"],
                                         ES, ES, 128, transpose=False,
                                         single_packet=False)
                    nc.gpsimd.dma_gather(gv[:], src_tbl[:], idx["vsg"],
                                         ES, ES, 128, transpose=False,
                                         single_packet=False)
                    if i == 0:
                        for gsrc, gdstT, gtag in ((gu, huT, "gTu"),
                                                  (gv, hvT, "gTv")):
                            gT = bigpool.tile([128, ES], BF16, tag=gtag)
                            for t in range(ET):
                                pst = ps_mmb.tile([128, 128], BF16, tag="mmb")
                                nc.tensor.transpose(pst[:], gsrc[:, t:t + 1, :],
                                                    identb[:])
                                nc.scalar.activation(gT[:, ts(t, 128)],
                                                     pst[:], AF.Copy)
                            for ch in range(2):
                                psh = ps_mm.tile([128, 512], F32, tag="mm")
                                nc.tensor.matmul(
                                    psh[:], fcnwT[:],
                                    gT[0:ND + 1, ts(ch, 512)],
                                    start=True, stop=True)
                                lrelu(psh[:], gdstT[:, ts(ch, 512)],
                                      [128, 512])
                    else:
                        for gsrc, gdstT in ((gu, huT), (gv, hvT)):
                            for t in range(ET):
                                pst = ps_mmb.tile([128, 128], BF16, tag="mmb")
                                nc.tensor.transpose(pst[:], gsrc[:, t:t + 1, :],
                                                    identb[:])
                                nc.scalar.activation(gdstT[:, ts(t, 128)],
                                                     pst[:], AF.Copy)

                    # ---- hid per e-tile -> x = p*hid (bf16) + S ----
                    xs = []
                    ps_ = []
                    psS = ps_s.tile([128, 1], F32, tag="s")
                    for t in range(ET):
                        psh = ps_mm.tile([128, 128], F32, tag="mm")
                        nc.tensor.matmul(psh[:], huT[:, ts(t, 128)],
                                         Wl("mwu", i), start=True, stop=False)
                        nc.tensor.matmul(psh[:], hvT[:, ts(t, 128)],
                                         Wl("mwv", i), start=False, stop=False)
                        nc.tensor.matmul(psh[:], eT[:, ts(t, 128)],
                                         Wl("mwe", i), start=False, stop=True)
                        hid_t = hidp.tile([128, 128], F32, tag=f"hid{t % 2}")
                        lrelu(psh[:], hid_t[:], [128, 128])
                        # S^T column: accumulate sum over edges of hid
                        nc.tensor.matmul(psS[:], hid_t[:], ones_col[:],
                                         start=(t == 0), stop=(t == ET - 1))
                        am = work.tile([128, 128], F32, tag="am")
                        nc.vector.tensor_mul(am[:], hid_t[:],
                                             mawB[:, ts(i, H)])
                        a_t = small.tile([128, 1], F32, tag="a_t")
                        nc.vector.reduce_sum(a_t[:], am[:],
                                             axis=mybir.AxisListType.X)
                        p_f = small.tile([128, 1], F32, tag="p_f")
                        nc.scalar.activation(p_f[:], a_t[:], AF.Exp,
                                             bias=mabB[:, i:i + 1])
                        p_t = small.tile([128, 1], BF16, tag=f"p_t{t}")
                        nc.scalar.activation(p_t[:], p_f[:], AF.Copy)
                        x_t = hidp.tile([128, 128], BF16, tag=f"x{t}")
                        nc.scalar.activation(x_t[:], hid_t[:], AF.Copy,
                                             scale=p_f[:])
                        xs.append(x_t)
                        ps_.append(p_t)

                    # ---- aggregation matmuls against the incidence shard:
                    # chunk ch of 512 nodes == stripe block of core ch ----
                    sT = small.tile([128, 1], BF16, tag="sT")
                    nc.scalar.activation(sT[:], psS[:], AF.Copy)
                    s8c = small.tile([128, 8], BF16, tag="s8c")
                    nc.vector.tensor_copy(s8c[:, 0:1], sT[:])
                    nc.vector.tensor_copy(s8c[:, 1:2], s8c[:, 0:1])
                    nc.vector.tensor_copy(s8c[:, 2:4], s8c[:, 0:2])
                    nc.vector.tensor_copy(s8c[:, 4:8], s8c[:, 0:4])
                    acc_b = acc[:].rearrange("(b r) c -> r b c", r=AC_R)
                    nc.sync.dma_start(acc_b[0:128, :, NS:NS + 1], s8c[:])
                    zrow = bigpool.tile([1, N], BF16, tag="zrow")
                    for half in range(2):
                        st4 = bigpool.tile([128, 4, 512], BF16, tag=f"st4_{half}")
                        for sub in range(4):
                            ch = half * 4 + sub
                            psa = ps_agg.tile([128, 512], F32, tag="agg")
                            psz = ps_s.tile([1, 512], F32, tag="s")
                            for t in range(ET):
                                nc.tensor.matmul(psa[:], xs[t][:],
                                                 nesT[:, t, ts(ch, 512)],
                                                 start=(t == 0),
                                                 stop=(t == ET - 1))
                                nc.tensor.matmul(psz[:], ps_[t][:],
                                                 nesT[:, t, ts(ch, 512)],
                                                 start=(t == 0),
                                                 stop=(t == ET - 1))
                            nc.scalar.activation(st4[:, sub, :], psa[:],
                                                 AF.Copy)
                            nc.vector.tensor_copy(zrow[:, ts(ch, 512)],
                                                  psz[:])
                        nc.sync.dma_start(
                            acc_b[0:128, half * 4:half * 4 + 4, 0:NS],
                            st4[:])
                    nc.sync.dma_start(
                        acc_b[128:129, :, 0:NS],
                        zrow[:].rearrange("p (b c) -> p b c", c=NS))
                    nc.gpsimd.collective_compute(
                        "ReduceScatter", ALU.add, replica_groups=RG,
                        ins=[acc[:]], outs=[acc_r[i][:]])

                    # ---- e' (skip on last layer: unused) ----
                    new_eT = None
                    if i < L - 1:
                        new_eT = bigpool.tile([HE + 1, ES], BF16,
                                              tag=f"eT_{(rep + i + 1) % 2}")
                        nc.vector.memset(new_eT[HE:HE + 1, :], 1.0)
                        for ch in range(2):
                            pse = ps_e.tile([HE, 512], F32, tag="e")
                            nc.tensor.matmul(pse[:], Wl("mewu", i),
                                             huT[:, ts(ch, 512)],
                                             start=True, stop=False)
                            nc.tensor.matmul(pse[:], Wl("mewv", i),
                                             hvT[:, ts(ch, 512)],
                                             start=False, stop=False)
                            nc.tensor.matmul(pse[:], Wl("mewe", i),
                                             eT[:, ts(ch, 512)],
                                             start=False, stop=True)
                            lrelu(pse[:], new_eT[0:HE, ts(ch, 512)],
                                  [HE, 512])

                    # ---- ctx stripe + GRU per node tile ----
                    ctxT = work.tile([128, NS], BF16, tag="ctxTs")
                    nc.sync.dma_start(ctxT[:], acc_r[i][0:128, 0:NS])
                    zcolb = small.tile([128, NT], BF16, tag="zcolb")
                    zcol = small.tile([128, NT], F32, tag="zcol")
                    nc.sync.dma_start(
                        zcolb[:],
                        acc_r[i][128:129, 0:NS].rearrange(
                            "o (t p) -> (o p) t", p=128))
                    nc.vector.tensor_copy(zcol[:], zcolb[:])
                    scol = small.tile([128, 1], BF16, tag="scol")
                    nc.sync.dma_start(scol[:], acc_r[i][0:128, NS:NS + 1])
                    ssc = small.tile([128, 1], BF16, tag="ssc")
                    nc.scalar.activation(ssc[:], scol[:], AF.Copy,
                                         scale=1.0 / E)
                    # SW = (S/E) @ wih.T  [1, 384]
                    psw = ps_g.tile([128, 3 * H], F32, tag="psgi")
                    nc.tensor.matmul(psw[0:1, :], ssc[:], Wl("gwih", i),
                                     start=True, stop=True)
                    SW = small.tile([1, 3 * H], F32, tag="SW")
                    nc.vector.tensor_copy(SW[:], psw[0:1, :])
                    SWb = work.tile([128, 3 * H], F32, tag="SWb")
                    bcast128(SWb[:], SW[:], 3 * H)

                    hT_new = bigpool.tile([H, NS], BF16,
                                          tag=f"hT_{(rep + i + 1) % 2}")
                    hbf_all = work.tile([128, NT, 128], BF16, tag="hbf_all")
                    h_np_new = []
                    for t in range(NT):
                        Zs = small.tile([128, 1], F32, tag="Zs")
                        nc.vector.tensor_scalar_max(Zs[:], zcol[:, t:t + 1],
                                                    1e-37)
                        rz = small.tile([128, 1], F32, tag="rz")
                        nc.vector.reciprocal(rz[:], Zs[:])
                        mk = small.tile([128, 1], F32, tag="mk")
                        nc.vector.tensor_scalar(mk[:], zcol[:, t:t + 1], 0.0,
                                                None, op0=ALU.is_equal)
                        psgi = ps_g.tile([128, 3 * H], F32, tag="psgi")
                        nc.tensor.matmul(psgi[:], ctxT[:, ts(t, 128)],
                                         Wl("gwih", i), start=True, stop=True)
                        gi = work.tile([128, 3 * H], F32, tag="gi")
                        nc.scalar.activation(gi[:], psgi[:], AF.Copy,
                                             scale=rz[:])
                        iso = work.tile([128, 3 * H], F32, tag="iso")
                        nc.vector.tensor_scalar_mul(iso[:], SWb[:], mk[:])
                        nc.vector.tensor_add(gi[:], gi[:], iso[:])
                        psgh = ps_g.tile([128, 3 * H], F32, tag="psgh")
                        nc.tensor.matmul(psgh[:], hT[:, ts(t, 128)],
                                         Wl("gwhh", i), start=True, stop=False)
                        nc.tensor.matmul(psgh[:], ones_row[:],
                                         gbh[:, ts(i, 3 * H)],
                                         start=False, stop=True)
                        rs_ = work.tile([128, 128], F32, tag="rsum")
                        nc.vector.tensor_add(rs_[:], psgh[:, 0:128],
                                             gi[:, 0:128])
                        r_t = work.tile([128, 128], F32, tag="r_t")
                        nc.scalar.activation(r_t[:], rs_[:], AF.Sigmoid)
                        zs_ = work.tile([128, 128], F32, tag="zsum")
                        nc.vector.tensor_add(zs_[:], psgh[:, 128:256],
                                             gi[:, 128:256])
                        z_t = work.tile([128, 128], F32, tag="z_t")
                        nc.scalar.activation(z_t[:], zs_[:], AF.Sigmoid)
                        rhn = work.tile([128, 128], F32, tag="rhn")
                        nc.vector.tensor_mul(rhn[:], r_t[:], psgh[:, 256:384])
                        nc.vector.tensor_add(rhn[:], rhn[:], gi[:, 256:384])
                        nc.vector.tensor_add(rhn[:], rhn[:],
                                             gbinB[:, ts(i, H)])
                        n_t = work.tile([128, 128], F32, tag="n_t")
                        nc.scalar.activation(n_t[:], rhn[:], AF.Tanh)
                        hmn = work.tile([128, 128], F32, tag="hmn")
                        nc.vector.tensor_sub(hmn[:], h_np[t][:], n_t[:])
                        nc.vector.tensor_mul(hmn[:], z_t[:], hmn[:])
                        h2 = hnpp.tile([128, 128], F32, tag=f"hnp{t}")
                        nc.vector.tensor_add(h2[:], n_t[:], hmn[:])
                        if i < L - 1:
                            nc.scalar.activation(h2[:], h2[:], AF.Relu)
                            nc.scalar.activation(hbf_all[:, t, :], h2[:],
                                                 AF.Copy)
                        pst2 = ps_mm.tile([128, 128], F32, tag="mm")
                        nc.tensor.transpose(pst2[:], h2[:], ident[:])
                        nc.scalar.activation(hT_new[:, ts(t, 128)], pst2[:],
                                             AF.Copy)
                        h_np_new.append(h2)

                    if i < L - 1:
                        nc.sync.dma_start(
                            agin[i][:].rearrange("(t p) f -> p t f", p=128),
                            hbf_all[:])
                        nc.gpsimd.collective_compute(
                            "AllGather", ALU.bypass, replica_groups=RG,
                            ins=[agin[i][:]], outs=[tables[i][:]])
                        eT = new_eT
                    hT = hT_new
                    h_np = h_np_new

                # ---------------- pooling ----------------
                ps_at = ps_s.tile([128, NT], F32, tag="s")
                expat = []
                for t in range(NT):
                    nc.tensor.matmul(ps_at[:, t:t + 1], hT[:, ts(t, 128)],
                                     paw[:], start=True, stop=True)
                    ea = small.tile([128, 1], F32, tag=f"expat{t}")
                    nc.scalar.activation(ea[:], ps_at[:, t:t + 1], AF.Exp,
                                         bias=pabB[:])
                    expat.append(ea)
                wtsu = []
                psZ = ps_e.tile([1, MS], F32, tag="e")
                for t in range(NT):
                    wu = work.tile([128, MS], F32, tag=f"wtsu{t}")
                    nc.scalar.activation(wu[:], mnTs[:, t, :], AF.Copy,
                                         scale=expat[t][:])
                    nc.tensor.matmul(psZ[:], ones_col[:], wu[:],
                                     start=(t == 0), stop=(t == NT - 1))
                    wtsu.append(wu)
                rZ = small.tile([1, MS], F32, tag="rZ")
                nc.vector.reciprocal(rZ[:], psZ[:])
                rZb = work.tile([128, MS], F32, tag="rZb")
                bcast128(rZb[:], rZ[:], MS)

                ps_ro = ps_g.tile([MS, H], F32, tag="psgi")
                wband = bigpool.tile([MS, NS], F32, tag="wband")
                for t in range(NT):
                    wf = work.tile([128, MS], F32, tag="wf")
                    nc.vector.tensor_mul(wf[:], wtsu[t][:], rZb[:])
                    psg = ps_mm.tile([128, H], F32, tag="mm")
                    nc.tensor.matmul(psg[:], hT[:, ts(t, 128)], pw[:],
                                     start=True, stop=False)
                    nc.tensor.matmul(psg[:], ones_row[:], pb[:],
                                     start=False, stop=True)
                    g_t = work.tile([128, H], F32, tag="g_t")
                    lrelu(psg[:], g_t[:], [128, H])
                    nc.tensor.matmul(ps_ro[:], wf[:], g_t[:],
                                     start=(t == 0), stop=(t == NT - 1))
                    pswt = ps_g.tile([MS, 128], F32, tag="psgh")
                    nc.tensor.transpose(pswt[:], wf[:], ident[:])
                    nc.scalar.activation(wband[:, ts(t, 128)], pswt[:],
                                         AF.Copy)

                ro_sb = work.tile([MS, H], F32, tag="ro_sb")
                nc.scalar.activation(ro_sb[:], ps_ro[:], AF.Copy)
                nc.sync.dma_start(out_ro[:], ro_sb[:])
                nc.sync.dma_start(out_wts[:], wband[:])

    nc.compile()
    return nc


# ----------------------------------------------------------------------------
# PJRT SPMD runner (inlined; no NTFF profiling available under this axon
# deployment).  The neuron NEFF cache fingerprints the HLO without the
# custom-call backend_config (where the BIR lives), so the jitted function
# carries an unused parameter whose shape encodes a hash of the BIR.
# ----------------------------------------------------------------------------
import hashlib
import jax
from jax.sharding import Mesh, PartitionSpec
from jax.experimental.shard_map import shard_map
from concourse import bass2jax


class _SpmdRunner:
    def __init__(self, nc, n_cores):
        bass2jax.install_neuronx_cc_hook()
        self.nc = nc
        self.n_cores = n_cores
        partition_name = (
            nc.partition_id_tensor.name if nc.partition_id_tensor else None
        )
        in_names, out_names, out_avals = [], [], []
        for alloc in nc.m.functions[0].allocations:
            if not isinstance(alloc, mybir.MemoryLocationSet):
                continue
            name = alloc.memorylocations[0].name
            if alloc.kind == "ExternalInput":
                if name != partition_name:
                    in_names.append(name)
            elif alloc.kind == "ExternalOutput":
                out_names.append(name)
                out_avals.append(jax.core.ShapedArray(
                    tuple(alloc.tensor_shape), mybir.dt.np(alloc.dtype)))
        self.in_names, self.out_names, self.out_avals = \
            in_names, out_names, out_avals
        n_params = len(in_names)
        all_in_names = list(in_names) + list(out_names)
        if partition_name is not None:
            all_in_names.append(partition_name)
        self._zero_outs = [
            np.zeros((n_cores * a.shape[0], *a.shape[1:]), a.dtype)
            for a in out_avals]
        bir_hash = int(hashlib.sha256(nc.to_json_bytes()).hexdigest()[:8], 16)
        self._salt = np.zeros((1 + bir_hash % 509,
                               1 + (bir_hash // 509) % 127), np.int8)

        def _body(*args):
            operands = list(args[:-1])
            if partition_name is not None:
                operands.append(bass2jax.partition_id_tensor())
            return tuple(bass2jax._bass_exec_p.bind(
                *operands,
                out_avals=tuple(out_avals),
                in_names=tuple(all_in_names),
                out_names=tuple(out_names),
                lowering_input_output_aliases=(),
                sim_require_finite=False,
                sim_require_nnan=False,
                nc=nc,
            ))

        devices = jax.devices()[:n_cores]
        mesh = Mesh(np.asarray(devices), ("core",))
        in_specs = (PartitionSpec("core"),) * (n_params + len(out_names)) + (
            PartitionSpec(),)
        out_specs = (PartitionSpec("core"),) * len(out_names)
        self._fn = jax.jit(
            shard_map(_body, mesh=mesh, in_specs=in_specs,
                      out_specs=out_specs, check_rep=False),
            keep_unused=True)

    def run(self, in_maps):
        n = self.n_cores
        concat = [
            np.concatenate([np.asarray(in_maps[c][k]) for c in range(n)], 0)
            for k in self.in_names]
        args = concat + self._zero_outs + [self._salt]
        outs = self._fn(*args)
        outs = [np.asarray(o) for o in outs]
        return [
            {name: outs[j].reshape(n, *self.out_avals[j].shape)[c]
             for j, name in enumerate(self.out_names)}
            for c in range(n)]


_CACHED = {}


def kernel(**inputs):
    """Full (unsharded) inputs -> full outputs (readout [256,128],
    wts [256,4096]), computed on 8 TRN2 NeuronCores."""
    if "nc" not in _CACHED:
        _CACHED["nc"] = build_kernel(reps=1)
        _CACHED["runner"] = _SpmdRunner(_CACHED["nc"], N_CORES)
    in_maps = prep_inputs(inputs)
    results = _CACHED["runner"].run(in_maps)
    return assemble_outputs(results)


# revision 3
# speedup vs baseline: 3.5274x; 2.2367x over previous
"""AMPNN (gnn_message_passing) distributed Bass kernel for 8 TRN2 cores.

Strategy (matches the problem's sharding hint): column-shard the
[n_node, n_edge] incidence matrix over edges, run the masked softmax +
aggregation as TensorE matmuls against the bf16 incidence shard held in
SBUF, and AllReduce the per-node context.

node_edge_mask = where(ne>0, 0, -1e6) makes the reference softmax an exact
per-node softmax over incident edges (non-incident terms underflow to 0),
so per edge we compute p = exp(a) and accumulate
  ctx_raw[:, n] = sum_e (p*hid)[e, :] * ne[n, e]      (128 x n matmul)
  Z[n]          = sum_e p[e] * ne[n, e]               (1 x n matmul)
with ne_shard^T [1024, 4096] as the moving operand (bf16, exact for 0/1).

- edges sharded 1024/core; nodes 512/core; mols 32/core
- gather h[us], h[vs] via gpsimd.dma_gather from a bf16 node table in DRAM
  (layer 0 gathers padded node_features instead and applies the input MLP
  per-edge: lrelu/gather commute); PE transposes to channel-partition
- per-edge MLPs as TensorE matmuls with channels on partitions (bf16)
- the accumulator is laid out as 8 row-blocks of [129, 513] (ctx^T
  channels + Z row, one block per node stripe, + an S column for the
  isolated-node uniform-softmax fallback), so ONE ReduceScatter per layer
  hands each core exactly its stripe with no core-dependent addressing
- the ReduceScattered ctx^T block is directly the GRU's lhsT (no
  transposes);
  normalization by 1/Z and the isolated-node fallback are applied to the
  matmul output with per-partition scales / rank-1 terms
- GRU on the local node stripe (bf16 matmuls, f32 gates/state); relu'd h
  AllGathered in bf16 as the next layer's gather table
- attentive pooling per mol stripe (16 contiguous nodes/mol); host embeds
  the [32, 512] wts band into the exact-zero [256, 4096] output
"""
import os
import numpy as np
import ml_dtypes
import concourse.bass as bass
import concourse.bacc as bacc
import concourse.tile as tile
import concourse.mybir as mybir

F32 = mybir.dt.float32
BF16 = mybir.dt.bfloat16
I16 = mybir.dt.int16
AF = mybir.ActivationFunctionType
ALU = mybir.AluOpType
ts = bass.ts

N_CORES = 8
N, E, M = 4096, 8192, 256
ND, ED, H, HE, C, L = 64, 16, 128, 64, 128, 3
ES = E // N_CORES          # 1024 edges/core
NS = N // N_CORES          # 512 nodes/core
MS = M // N_CORES          # 32 mols/core
ET = ES // 128             # 8 edge tiles
NT = NS // 128             # 4 node tiles
AC_R = 129                 # block rows: 128 ctx channels + Z row
AC_C = NS + 1              # block cols: 512 stripe nodes + S column
RG = [list(range(N_CORES))]


def wrap16(a):
    """[n] -> [128, n//16]: 16-partition wrap (unwrapped[i] ==
    wrapped[i % 16, i // 16]) replicated to all 128 partitions (the 8
    gpsimd Q7 cores each read their own 16-partition copy)."""
    w = np.ascontiguousarray(a.reshape(-1, 16).T)
    return np.tile(w, (8, 1))


def bf(x):
    return np.ascontiguousarray(x).astype(ml_dtypes.bfloat16)


def prep_inputs(inputs):
    """Full problem inputs -> per-core in_maps (host-side shard/layout)."""
    nf = np.asarray(inputs["node_features"], np.float32)
    ef = np.asarray(inputs["edge_features"], np.float32)
    us = np.asarray(inputs["us"], np.int64)
    vs = np.asarray(inputs["vs"], np.int64)
    mn = np.asarray(inputs["mol_node_matrix"], np.float32)
    ne = np.asarray(inputs["node_edge_matrix"], np.float32)
    p = {k: np.asarray(inputs[k], np.float32) for k in (
        "fcn_w", "fcn_b", "fce_w", "fce_b", "m_w", "m_b", "me_w", "me_b",
        "ma_w", "ma_b", "g_wih", "g_whh", "g_bih", "g_bhh",
        "pool_at_w", "pool_at_b", "pool_w", "pool_b")}

    nfpad = np.zeros((N, 128), np.float32)
    nfpad[:, :ND] = nf
    nfpad[:, ND] = 1.0

    fcnwT = np.concatenate([p["fcn_w"].T, p["fcn_b"][None, :]], 0)   # [65,128]
    fcewT = np.concatenate([p["fce_w"].T, p["fce_b"][None, :]], 0)   # [17,64]
    hs = np.hstack
    mwu = hs([p["m_w"][i][:, 0:H].T for i in range(L)])              # [128,3*128]
    mwv = hs([p["m_w"][i][:, H:2 * H].T for i in range(L)])
    mwe = hs([np.concatenate([p["m_w"][i][:, 2 * H:].T,
                              p["m_b"][i][None, :]], 0) for i in range(L)])
    mewu = hs([p["me_w"][i][:, 0:H].T for i in range(L)])            # [128,3*64]
    mewv = hs([p["me_w"][i][:, H:2 * H].T for i in range(L)])
    mewe = hs([np.concatenate([p["me_w"][i][:, 2 * H:].T,
                               p["me_b"][i][None, :]], 0) for i in range(L)])
    gwih = hs([p["g_wih"][i].T for i in range(L)])                   # [128,3*384]
    gwhh = hs([p["g_whh"][i].T for i in range(L)])
    # r,z bias slices can ride on the h-side rank-1 bias matmul; the n-slice
    # of g_bih must be added after the r*hn product, so it goes separately.
    gbh = np.concatenate([
        np.concatenate([p["g_bih"][i][0:2 * H] + p["g_bhh"][i][0:2 * H],
                        p["g_bhh"][i][2 * H:]]) for i in range(L)])
    gbin = hs([p["g_bih"][i][None, 2 * H:] for i in range(L)])       # [1,3*128]

    common = {
        "nfpad": bf(nfpad),
        "fcnwT": bf(fcnwT), "fcewT": bf(fcewT),
        "mwu": bf(mwu), "mwv": bf(mwv), "mwe": bf(mwe),
        "mewu": bf(mewu), "mewv": bf(mewv), "mewe": bf(mewe),
        "maw": np.ascontiguousarray(p["ma_w"].reshape(1, L * C)),
        "mab": np.ascontiguousarray(p["ma_b"].reshape(1, L)),
        "gwih": bf(gwih), "gwhh": bf(gwhh),
        "gbh": bf(gbh.reshape(1, L * 3 * H)),
        "gbin": np.ascontiguousarray(gbin),                          # f32
        "paw": bf(p["pool_at_w"].T),                                 # [128,1]
        "pab": p["pool_at_b"].reshape(1, 1).astype(np.float32),
        "pw": bf(p["pool_w"].T),                                     # [128,128]
        "pb": bf(p["pool_b"].reshape(1, H)),
        "identf": np.eye(128, dtype=np.float32),
        "identb": np.eye(128).astype(ml_dtypes.bfloat16),
    }

    in_maps = []
    for c in range(N_CORES):
        el, eh = c * ES, (c + 1) * ES
        nl, nh = c * NS, (c + 1) * NS
        ml, mh = c * MS, (c + 1) * MS
        us_s, vs_s = us[el:eh], vs[el:eh]
        nfTs = np.concatenate([nf[nl:nh].T, np.ones((1, NS), np.float32)], 0)
        efTs = np.concatenate([ef[el:eh].T, np.ones((1, ES), np.float32)], 0)
        m = dict(common)
        m.update({
            "nfTs": bf(nfTs),                          # [65, 512]
            "efTs": bf(efTs),                          # [17, 1024]
            "idxall": np.ascontiguousarray(np.concatenate(
                [wrap16(us_s.astype(np.int16)),
                 wrap16(vs_s.astype(np.int16))], axis=1)),
            "mnTs": np.ascontiguousarray(mn[ml:mh, nl:nh].T),  # [512, 32]
            # incidence shard transposed, as [128, ET, N] bf16 (0/1 exact)
            "nesT": bf(np.ascontiguousarray(
                ne[:, el:eh].T.reshape(ET, 128, N).transpose(1, 0, 2))),
        })
        in_maps.append(m)
    return in_maps


def assemble_outputs(results):
    readout = np.concatenate([results[c]["ro"] for c in range(N_CORES)], 0)
    wts = np.zeros((M, N), np.float32)
    for c in range(N_CORES):
        wts[c * MS:(c + 1) * MS, c * NS:(c + 1) * NS] = results[c]["wts"]
    return readout, wts


def build_kernel(plan=None, reps=1):
    nc = bacc.Bacc("TRN2", target_bir_lowering=False, debug=False,
                   num_devices=N_CORES)

    din = {}

    def inp(name, shape, dt):
        din[name] = nc.dram_tensor(name, list(shape), dt, kind="ExternalInput")
        return din[name]

    inp("nfpad", (N, 128), BF16)
    inp("nfTs", (ND + 1, NS), BF16)
    inp("efTs", (ED + 1, ES), BF16)
    inp("fcnwT", (ND + 1, H), BF16)
    inp("fcewT", (ED + 1, HE), BF16)
    inp("mwu", (H, L * H), BF16); inp("mwv", (H, L * H), BF16)
    inp("mwe", (HE + 1, L * H), BF16)
    inp("mewu", (H, L * HE), BF16); inp("mewv", (H, L * HE), BF16)
    inp("mewe", (HE + 1, L * HE), BF16)
    inp("maw", (1, L * H), F32); inp("mab", (1, L), F32)
    inp("gwih", (H, L * 3 * H), BF16); inp("gwhh", (H, L * 3 * H), BF16)
    inp("gbh", (1, L * 3 * H), BF16)
    inp("gbin", (1, L * H), F32)
    inp("paw", (H, 1), BF16); inp("pab", (1, 1), F32)
    inp("pw", (H, H), BF16); inp("pb", (1, H), BF16)
    inp("mnTs", (NS, MS), F32)
    inp("identf", (128, 128), F32)
    inp("identb", (128, 128), BF16)
    inp("idxall", (128, 2 * (ES // 16)), I16)
    inp("nesT", (128, ET, N), BF16)

    out_ro = nc.dram_tensor("ro", [MS, H], F32, kind="ExternalOutput")
    out_wts = nc.dram_tensor("wts", [MS, NS], F32, kind="ExternalOutput")

    acc = nc.dram_tensor("acc", [N_CORES * AC_R, AC_C], BF16,
                         kind="Internal")
    acc_r = [nc.dram_tensor(f"accr{i}", [AC_R, AC_C], BF16, kind="Internal")
             for i in range(L)]
    tables = [nc.dram_tensor(f"table{i}", [N, H], BF16, kind="Internal",
                             addr_space="Shared") for i in (1, 2)]
    agin = [nc.dram_tensor(f"agin{i}", [NS, H], BF16, kind="Internal")
            for i in (1, 2)]

    with tile.TileContext(nc) as tc:
        with tc.tile_pool(name="const", bufs=1) as cpool, \
             tc.tile_pool(name="wts_sb", bufs=1) as wpool, \
             tc.tile_pool(name="big", bufs=1) as bigpool, \
             tc.tile_pool(name="work", bufs=2) as work, \
             tc.tile_pool(name="hid", bufs=2) as hidp, \
             tc.tile_pool(name="hnp", bufs=2) as hnpp, \
             tc.tile_pool(name="small", bufs=4) as small, \
             tc.tile_pool(name="ps_mm", bufs=2, space="PSUM") as ps_mm, \
             tc.tile_pool(name="ps_mmb", bufs=1, space="PSUM") as ps_mmb, \
             tc.tile_pool(name="ps_agg", bufs=1, space="PSUM") as ps_agg, \
             tc.tile_pool(name="ps_e", bufs=1, space="PSUM") as ps_e, \
             tc.tile_pool(name="ps_s", bufs=1, space="PSUM") as ps_s, \
             tc.tile_pool(name="ps_g", bufs=1, space="PSUM") as ps_g:

            def lrelu(psum_ap, out_ap, shape):
                tmp = work.tile(shape, F32, tag=f"lrt{shape[0]}x{shape[1]}")
                nc.scalar.activation(tmp[:], psum_ap, AF.Copy, scale=0.01)
                nc.vector.tensor_max(out_ap, psum_ap, tmp[:])

            ones_col = cpool.tile([128, 1], F32)
            nc.vector.memset(ones_col[:], 1.0)
            ones_colb = cpool.tile([128, 1], BF16)
            nc.vector.memset(ones_colb[:], 1.0)
            ones_row = cpool.tile([1, 128], BF16)
            nc.vector.memset(ones_row[:], 1.0)
            ones_rowf = cpool.tile([1, 128], F32)
            nc.vector.memset(ones_rowf[:], 1.0)

            def load(name, shape, dt):
                t = wpool.tile(shape, dt, tag=name)
                nc.sync.dma_start(t[:], din[name][:])
                return t

            idxall = load("idxall", [128, 2 * (ES // 16)], I16)
            iw = ES // 16
            idx = {"usg": idxall[:, 0:iw], "vsg": idxall[:, iw:2 * iw]}
            fcnwT = load("fcnwT", [ND + 1, H], BF16)
            ident = load("identf", [128, 128], F32)
            identb = load("identb", [128, 128], BF16)
            fcewT = load("fcewT", [ED + 1, HE], BF16)
            nfTs = load("nfTs", [ND + 1, NS], BF16)
            efTs = load("efTs", [ED + 1, ES], BF16)
            nesT = load("nesT", [128, ET, N], BF16)
            W = {}
            for nm, r, cdim in (
                ("mwu", H, H), ("mwv", H, H), ("mwe", HE + 1, H),
                ("mewu", H, HE), ("mewv", H, HE), ("mewe", HE + 1, HE),
                ("gwih", H, 3 * H), ("gwhh", H, 3 * H),
            ):
                W[nm] = (load(nm, [r, L * cdim], BF16), cdim)
            gbh = load("gbh", [1, L * 3 * H], BF16)
            gbin = load("gbin", [1, L * H], F32)
            maw_r = load("maw", [1, L * H], F32)
            mab_r = load("mab", [1, L], F32)
            pab_r = load("pab", [1, 1], F32)
            paw = load("paw", [H, 1], BF16)
            pw = load("pw", [H, H], BF16)
            pb = load("pb", [1, H], BF16)
            mnTs = wpool.tile([128, NT, MS], F32, tag="mnTs")
            for t in range(NT):
                nc.sync.dma_start(mnTs[:, t, :], din["mnTs"][ts(t, 128), :])

            def bcast128(dst_ap, src_row_ap, width, nparts=128):
                pbc = ps_mm.tile([128, 512], F32, tag="mm")
                nc.tensor.matmul(pbc[:nparts, 0:width],
                                 ones_rowf[:, 0:nparts], src_row_ap,
                                 start=True, stop=True)
                nc.scalar.activation(dst_ap, pbc[:nparts, 0:width], AF.Copy)

            mawB = cpool.tile([128, L * H], F32)
            bcast128(mawB[:], maw_r[:], L * H)
            mabB = cpool.tile([128, L], F32)
            bcast128(mabB[:], mab_r[:], L)
            pabB = cpool.tile([128, 1], F32)
            bcast128(pabB[:], pab_r[:], 1)
            gbinB = cpool.tile([128, L * H], F32)
            bcast128(gbinB[:], gbin[:], L * H)

            def Wl(nm, i):
                t, cdim = W[nm]
                return t[:, ts(i, cdim)]

            for rep in range(reps):
                # ---------- h0 on own node stripe: hT [128, 512] bf16 -----
                hT = bigpool.tile([H, NS], BF16, tag=f"hT_{rep % 2}")
                ps0 = ps_mm.tile([128, NS], F32, tag="mm")
                nc.tensor.matmul(ps0[:], fcnwT[:], nfTs[:], start=True,
                                 stop=True)
                lrelu(ps0[:], hT[:], [H, NS])
                h_np = []
                for t in range(NT):
                    pst = ps_mmb.tile([128, 128], BF16, tag="mmb")
                    nc.tensor.transpose(pst[:], hT[:, ts(t, 128)], identb[:])
                    ht = hnpp.tile([128, 128], F32, tag=f"hnp{t}")
                    nc.scalar.activation(ht[:], pst[:], AF.Copy)
                    h_np.append(ht)

                # ---------- e0: eT [65, 1024] bf16 ------------------------
                eT = bigpool.tile([HE + 1, ES], BF16, tag=f"eT_{rep % 2}")
                nc.vector.memset(eT[HE:HE + 1, :], 1.0)
                for ch in range(2):
                    pse = ps_e.tile([HE, 512], F32, tag="e")
                    nc.tensor.matmul(pse[:], fcewT[:], efTs[:, ts(ch, 512)],
                                     start=True, stop=True)
                    lrelu(pse[:], eT[0:HE, ts(ch, 512)], [HE, 512])

                huT = bigpool.tile([128, ES], BF16, tag="huT")
                hvT = bigpool.tile([128, ES], BF16, tag="hvT")
                gu = bigpool.tile([128, ET, 128], BF16, tag="gu")
                gv = bigpool.tile([128, ET, 128], BF16, tag="gv")

                for i in range(L):
                    # ---- gather endpoint rows (edge-partition tiles) ----
                    src_tbl = din["nfpad"] if i == 0 else tables[i - 1]
                    nc.gpsimd.dma_gather(gu[:], src_tbl[:], idx["usg"],
                                         ES, ES, 128, transpose=False,
                                         single_packet=False)
                    nc.gpsimd.dma_gather(gv[:], src_tbl[:], idx["vsg"],
                                         ES, ES, 128, transpose=False,
                                         single_packet=False)
                    if i == 0:
                        for gsrc, gdstT, gtag in ((gu, huT, "gTu"),
                                                  (gv, hvT, "gTv")):
                            gT = bigpool.tile([128, ES], BF16, tag=gtag)
                            for t in range(ET):
                                pst = ps_mmb.tile([128, 128], BF16, tag="mmb")
                                nc.tensor.transpose(pst[:], gsrc[:, t:t + 1, :],
                                                    identb[:])
                                nc.scalar.activation(gT[:, ts(t, 128)],
                                                     pst[:], AF.Copy)
                            for ch in range(2):
                                psh = ps_mm.tile([128, 512], F32, tag="mm")
                                nc.tensor.matmul(
                                    psh[:], fcnwT[:],
                                    gT[0:ND + 1, ts(ch, 512)],
                                    start=True, stop=True)
                                lrelu(psh[:], gdstT[:, ts(ch, 512)],
                                      [128, 512])
                    else:
                        for gsrc, gdstT in ((gu, huT), (gv, hvT)):
                            for t in range(ET):
                                pst = ps_mmb.tile([128, 128], BF16, tag="mmb")
                                nc.tensor.transpose(pst[:], gsrc[:, t:t + 1, :],
                                                    identb[:])
                                nc.scalar.activation(gdstT[:, ts(t, 128)],
                                                     pst[:], AF.Copy)

                    # ---- hid per e-tile -> x = p*hid (bf16) + S ----
                    xs = []
                    ps_ = []
                    psS = ps_s.tile([128, 1], F32, tag="s")
                    for t in range(ET):
                        psh = ps_mm.tile([128, 128], F32, tag="mm")
                        nc.tensor.matmul(psh[:], huT[:, ts(t, 128)],
                                         Wl("mwu", i), start=True, stop=False)
                        nc.tensor.matmul(psh[:], hvT[:, ts(t, 128)],
                                         Wl("mwv", i), start=False, stop=False)
                        nc.tensor.matmul(psh[:], eT[:, ts(t, 128)],
                                         Wl("mwe", i), start=False, stop=True)
                        hid_t = hidp.tile([128, 128], F32, tag=f"hid{t % 2}")
                        lrelu(psh[:], hid_t[:], [128, 128])
                        # S^T column: accumulate sum over edges of hid
                        nc.tensor.matmul(psS[:], hid_t[:], ones_col[:],
                                         start=(t == 0), stop=(t == ET - 1))
                        am = work.tile([128, 128], F32, tag="am")
                        nc.vector.tensor_mul(am[:], hid_t[:],
                                             mawB[:, ts(i, H)])
                        a_t = small.tile([128, 1], F32, tag="a_t")
                        nc.vector.reduce_sum(a_t[:], am[:],
                                             axis=mybir.AxisListType.X)
                        p_f = small.tile([128, 1], F32, tag="p_f")
                        nc.scalar.activation(p_f[:], a_t[:], AF.Exp,
                                             bias=mabB[:, i:i + 1])
                        p_t = small.tile([128, 1], BF16, tag=f"p_t{t}")
                        nc.scalar.activation(p_t[:], p_f[:], AF.Copy)
                        x_t = hidp.tile([128, 128], BF16, tag=f"x{t}")
                        nc.scalar.activation(x_t[:], hid_t[:], AF.Copy,
                                             scale=p_f[:])
                        xs.append(x_t)
                        ps_.append(p_t)

                    # ---- aggregation matmuls against the incidence shard:
                    # chunk ch of 512 nodes == stripe block of core ch ----
                    sT = small.tile([128, 1], BF16, tag="sT")
                    nc.scalar.activation(sT[:], psS[:], AF.Copy)
                    s8c = small.tile([128, 8], BF16, tag="s8c")
                    nc.vector.tensor_copy(s8c[:, 0:1], sT[:])
                    nc.vector.tensor_copy(s8c[:, 1:2], s8c[:, 0:1])
                    nc.vector.tensor_copy(s8c[:, 2:4], s8c[:, 0:2])
                    nc.vector.tensor_copy(s8c[:, 4:8], s8c[:, 0:4])
                    acc_b = acc[:].rearrange("(b r) c -> r b c", r=AC_R)
                    nc.sync.dma_start(acc_b[0:128, :, NS:NS + 1], s8c[:])
                    zrow = bigpool.tile([1, N], BF16, tag="zrow")
                    for half in range(2):
                        st4 = bigpool.tile([128, 4, 512], BF16, tag=f"st4_{half}")
                        for sub in range(4):
                            ch = half * 4 + sub
                            psa = ps_agg.tile([128, 512], F32, tag="agg")
                            psz = ps_s.tile([1, 512], F32, tag="s")
                            for t in range(ET):
                                nc.tensor.matmul(psa[:], xs[t][:],
                                                 nesT[:, t, ts(ch, 512)],
                                                 start=(t == 0),
                                                 stop=(t == ET - 1))
                                nc.tensor.matmul(psz[:], ps_[t][:],
                                                 nesT[:, t, ts(ch, 512)],
                                                 start=(t == 0),
                                                 stop=(t == ET - 1))
                            nc.scalar.activation(st4[:, sub, :], psa[:],
                                                 AF.Copy)
                            nc.vector.tensor_copy(zrow[:, ts(ch, 512)],
                                                  psz[:])
                        nc.sync.dma_start(
                            acc_b[0:128, half * 4:half * 4 + 4, 0:NS],
                            st4[:])
                    nc.sync.dma_start(
                        acc_b[128:129, :, 0:NS],
                        zrow[:].rearrange("p (b c) -> p b c", c=NS))
                    nc.gpsimd.collective_compute(
                        "ReduceScatter", ALU.add, replica_groups=RG,
                        ins=[acc[:]], outs=[acc_r[i][:]])

                    # ---- e' (skip on last layer: unused) ----
                    new_eT = None
                    if i < L - 1:
                        new_eT = bigpool.tile([HE + 1, ES], BF16,
                                              tag=f"eT_{(rep + i + 1) % 2}")
                        nc.vector.memset(new_eT[HE:HE + 1, :], 1.0)
                        for ch in range(2):
                            pse = ps_e.tile([HE, 512], F32, tag="e")
                            nc.tensor.matmul(pse[:], Wl("mewu", i),
                                             huT[:, ts(ch, 512)],
                                             start=True, stop=False)
                            nc.tensor.matmul(pse[:], Wl("mewv", i),
                                             hvT[:, ts(ch, 512)],
                                             start=False, stop=False)
                            nc.tensor.matmul(pse[:], Wl("mewe", i),
                                             eT[:, ts(ch, 512)],
                                             start=False, stop=True)
                            lrelu(pse[:], new_eT[0:HE, ts(ch, 512)],
                                  [HE, 512])

                    # ---- ctx stripe + GRU per node tile ----
                    ctxT = work.tile([128, NS], BF16, tag="ctxTs")
                    nc.sync.dma_start(ctxT[:], acc_r[i][0:128, 0:NS])
                    zcolb = small.tile([128, NT], BF16, tag="zcolb")
                    zcol = small.tile([128, NT], F32, tag="zcol")
                    nc.sync.dma_start(
                        zcolb[:],
                        acc_r[i][128:129, 0:NS].rearrange(
                            "o (t p) -> (o p) t", p=128))
                    nc.vector.tensor_copy(zcol[:], zcolb[:])
                    scol = small.tile([128, 1], BF16, tag="scol")
                    nc.sync.dma_start(scol[:], acc_r[i][0:128, NS:NS + 1])
                    ssc = small.tile([128, 1], BF16, tag="ssc")
                    nc.scalar.activation(ssc[:], scol[:], AF.Copy,
                                         scale=1.0 / E)
                    # SW = (S/E) @ wih.T  [1, 384]
                    psw = ps_g.tile([128, 3 * H], F32, tag="psgi")
                    nc.tensor.matmul(psw[0:1, :], ssc[:], Wl("gwih", i),
                                     start=True, stop=True)
                    SW = small.tile([1, 3 * H], F32, tag="SW")
                    nc.vector.tensor_copy(SW[:], psw[0:1, :])
                    SWb = work.tile([128, 3 * H], F32, tag="SWb")
                    bcast128(SWb[:], SW[:], 3 * H)

                    hT_new = bigpool.tile([H, NS], BF16,
                                          tag=f"hT_{(rep + i + 1) % 2}")
                    hbf_all = work.tile([128, NT, 128], BF16, tag="hbf_all")
                    h_np_new = []
                    for t in range(NT):
                        Zs = small.tile([128, 1], F32, tag="Zs")
                        nc.vector.tensor_scalar_max(Zs[:], zcol[:, t:t + 1],
                                                    1e-37)
                        rz = small.tile([128, 1], F32, tag="rz")
                        nc.vector.reciprocal(rz[:], Zs[:])
                        mk = small.tile([128, 1], F32, tag="mk")
                        nc.vector.tensor_scalar(mk[:], zcol[:, t:t + 1], 0.0,
                                                None, op0=ALU.is_equal)
                        psgi = ps_g.tile([128, 3 * H], F32, tag="psgi")
                        nc.tensor.matmul(psgi[:], ctxT[:, ts(t, 128)],
                                         Wl("gwih", i), start=True, stop=True)
                        gi = work.tile([128, 3 * H], F32, tag="gi")
                        nc.scalar.activation(gi[:], psgi[:], AF.Copy,
                                             scale=rz[:])
                        iso = work.tile([128, 3 * H], F32, tag="iso")
                        nc.vector.tensor_scalar_mul(iso[:], SWb[:], mk[:])
                        nc.vector.tensor_add(gi[:], gi[:], iso[:])
                        psgh = ps_g.tile([128, 3 * H], F32, tag="psgh")
                        nc.tensor.matmul(psgh[:], hT[:, ts(t, 128)],
                                         Wl("gwhh", i), start=True, stop=False)
                        nc.tensor.matmul(psgh[:], ones_row[:],
                                         gbh[:, ts(i, 3 * H)],
                                         start=False, stop=True)
                        rs_ = work.tile([128, 128], F32, tag="rsum")
                        nc.vector.tensor_add(rs_[:], psgh[:, 0:128],
                                             gi[:, 0:128])
                        r_t = work.tile([128, 128], F32, tag="r_t")
                        nc.scalar.activation(r_t[:], rs_[:], AF.Sigmoid)
                        zs_ = work.tile([128, 128], F32, tag="zsum")
                        nc.vector.tensor_add(zs_[:], psgh[:, 128:256],
                                             gi[:, 128:256])
                        z_t = work.tile([128, 128], F32, tag="z_t")
                        nc.scalar.activation(z_t[:], zs_[:], AF.Sigmoid)
                        rhn = work.tile([128, 128], F32, tag="rhn")
                        nc.vector.tensor_mul(rhn[:], r_t[:], psgh[:, 256:384])
                        nc.vector.tensor_add(rhn[:], rhn[:], gi[:, 256:384])
                        nc.vector.tensor_add(rhn[:], rhn[:],
                                             gbinB[:, ts(i, H)])
                        n_t = work.tile([128, 128], F32, tag="n_t")
                        nc.scalar.activation(n_t[:], rhn[:], AF.Tanh)
                        hmn = work.tile([128, 128], F32, tag="hmn")
                        nc.vector.tensor_sub(hmn[:], h_np[t][:], n_t[:])
                        nc.vector.tensor_mul(hmn[:], z_t[:], hmn[:])
                        h2 = hnpp.tile([128, 128], F32, tag=f"hnp{t}")
                        nc.vector.tensor_add(h2[:], n_t[:], hmn[:])
                        if i < L - 1:
                            nc.scalar.activation(h2[:], h2[:], AF.Relu)
                            nc.scalar.activation(hbf_all[:, t, :], h2[:],
                                                 AF.Copy)
                        pst2 = ps_mm.tile([128, 128], F32, tag="mm")
                        nc.tensor.transpose(pst2[:], h2[:], ident[:])
                        nc.scalar.activation(hT_new[:, ts(t, 128)], pst2[:],
                                             AF.Copy)
                        h_np_new.append(h2)

                    if i < L - 1:
                        nc.sync.dma_start(
                            agin[i][:].rearrange("(t p) f -> p t f", p=128),
                            hbf_all[:])
                        nc.gpsimd.collective_compute(
                            "AllGather", ALU.bypass, replica_groups=RG,
                            ins=[agin[i][:]], outs=[tables[i][:]])
                        eT = new_eT
                    hT = hT_new
                    h_np = h_np_new

                # ---------------- pooling ----------------
                ps_at = ps_s.tile([128, NT], F32, tag="s")
                expat = []
                for t in range(NT):
                    nc.tensor.matmul(ps_at[:, t:t + 1], hT[:, ts(t, 128)],
                                     paw[:], start=True, stop=True)
                    ea = small.tile([128, 1], F32, tag=f"expat{t}")
                    nc.scalar.activation(ea[:], ps_at[:, t:t + 1], AF.Exp,
                                         bias=pabB[:])
                    expat.append(ea)
                wtsu = []
                psZ = ps_e.tile([1, MS], F32, tag="e")
                for t in range(NT):
                    wu = work.tile([128, MS], F32, tag=f"wtsu{t}")
                    nc.scalar.activation(wu[:], mnTs[:, t, :], AF.Copy,
                                         scale=expat[t][:])
                    nc.tensor.matmul(psZ[:], ones_col[:], wu[:],
                                     start=(t == 0), stop=(t == NT - 1))
                    wtsu.append(wu)
                rZ = small.tile([1, MS], F32, tag="rZ")
                nc.vector.reciprocal(rZ[:], psZ[:])
                rZb = work.tile([128, MS], F32, tag="rZb")
                bcast128(rZb[:], rZ[:], MS)

                ps_ro = ps_g.tile([MS, H], F32, tag="psgi")
                wband = bigpool.tile([MS, NS], F32, tag="wband")
                for t in range(NT):
                    wf = work.tile([128, MS], F32, tag="wf")
                    nc.vector.tensor_mul(wf[:], wtsu[t][:], rZb[:])
                    psg = ps_mm.tile([128, H], F32, tag="mm")
                    nc.tensor.matmul(psg[:], hT[:, ts(t, 128)], pw[:],
                                     start=True, stop=False)
                    nc.tensor.matmul(psg[:], ones_row[:], pb[:],
                                     start=False, stop=True)
                    g_t = work.tile([128, H], F32, tag="g_t")
                    lrelu(psg[:], g_t[:], [128, H])
                    nc.tensor.matmul(ps_ro[:], wf[:], g_t[:],
                                     start=(t == 0), stop=(t == NT - 1))
                    pswt = ps_g.tile([MS, 128], F32, tag="psgh")
                    nc.tensor.transpose(pswt[:], wf[:], ident[:])
                    nc.scalar.activation(wband[:, ts(t, 128)], pswt[:],
                                         AF.Copy)

                ro_sb = work.tile([MS, H], F32, tag="ro_sb")
                nc.scalar.activation(ro_sb[:], ps_ro[:], AF.Copy)
                nc.sync.dma_start(out_ro[:], ro_sb[:])
                nc.sync.dma_start(out_wts[:], wband[:])

    nc.compile()
    return nc


# ----------------------------------------------------------------------------
# PJRT SPMD runner (inlined; no NTFF profiling available under this axon
# deployment).  The neuron NEFF cache fingerprints the HLO without the
# custom-call backend_config (where the BIR lives), so the jitted function
# carries an unused parameter whose shape encodes a hash of the BIR.
# ----------------------------------------------------------------------------
import hashlib
import jax
from jax.sharding import Mesh, PartitionSpec
from jax.experimental.shard_map import shard_map
from concourse import bass2jax


class _SpmdRunner:
    def __init__(self, nc, n_cores):
        bass2jax.install_neuronx_cc_hook()
        self.nc = nc
        self.n_cores = n_cores
        partition_name = (
            nc.partition_id_tensor.name if nc.partition_id_tensor else None
        )
        in_names, out_names, out_avals = [], [], []
        for alloc in nc.m.functions[0].allocations:
            if not isinstance(alloc, mybir.MemoryLocationSet):
                continue
            name = alloc.memorylocations[0].name
            if alloc.kind == "ExternalInput":
                if name != partition_name:
                    in_names.append(name)
            elif alloc.kind == "ExternalOutput":
                out_names.append(name)
                out_avals.append(jax.core.ShapedArray(
                    tuple(alloc.tensor_shape), mybir.dt.np(alloc.dtype)))
        self.in_names, self.out_names, self.out_avals = \
            in_names, out_names, out_avals
        n_params = len(in_names)
        all_in_names = list(in_names) + list(out_names)
        if partition_name is not None:
            all_in_names.append(partition_name)
        self._zero_outs = [
            np.zeros((n_cores * a.shape[0], *a.shape[1:]), a.dtype)
            for a in out_avals]
        bir_hash = int(hashlib.sha256(nc.to_json_bytes()).hexdigest()[:8], 16)
        self._salt = np.zeros((1 + bir_hash % 509,
                               1 + (bir_hash // 509) % 127), np.int8)

        def _body(*args):
            operands = list(args[:-1])
            if partition_name is not None:
                operands.append(bass2jax.partition_id_tensor())
            return tuple(bass2jax._bass_exec_p.bind(
                *operands,
                out_avals=tuple(out_avals),
                in_names=tuple(all_in_names),
                out_names=tuple(out_names),
                lowering_input_output_aliases=(),
                sim_require_finite=False,
                sim_require_nnan=False,
                nc=nc,
            ))

        devices = jax.devices()[:n_cores]
        mesh = Mesh(np.asarray(devices), ("core",))
        in_specs = (PartitionSpec("core"),) * (n_params + len(out_names)) + (
            PartitionSpec(),)
        out_specs = (PartitionSpec("core"),) * len(out_names)
        self._fn = jax.jit(
            shard_map(_body, mesh=mesh, in_specs=in_specs,
                      out_specs=out_specs, check_rep=False),
            keep_unused=True)

    def stage(self, in_maps):
        n = self.n_cores
        concat = [
            np.concatenate([np.asarray(in_maps[c][k]) for c in range(n)], 0)
            for k in self.in_names]
        args = concat + self._zero_outs + [self._salt]
        self._dev_args = [jax.device_put(a) for a in args]
        jax.block_until_ready(self._dev_args)

    def run_raw(self):
        return self._fn(*self._dev_args)

    def run(self, in_maps):
        n = self.n_cores
        self.stage(in_maps)
        outs = [np.asarray(o) for o in self.run_raw()]
        return [
            {name: outs[j].reshape(n, *self.out_avals[j].shape)[c]
             for j, name in enumerate(self.out_names)}
            for c in range(n)]


_CACHED = {}


def kernel(**inputs):
    """Full (unsharded) inputs -> full outputs (readout [256,128],
    wts [256,4096]), computed on 8 TRN2 NeuronCores."""
    if "nc" not in _CACHED:
        _CACHED["nc"] = build_kernel(reps=1)
        _CACHED["runner"] = _SpmdRunner(_CACHED["nc"], N_CORES)
    in_maps = prep_inputs(inputs)
    results = _CACHED["runner"].run(in_maps)
    return assemble_outputs(results)
